# revision 1
# baseline (speedup 1.0000x reference)
"""Trainium2 Bass kernel for nn_CrossAttentionModule.

Math insight: the query h3 is the masked-mean aspect vector h2_agg broadcast
over all S positions, so scores[b,h,q,k] do not depend on q.  The whole
[B,S,S] output is a single row row[b,k] broadcast along the q axis:

    qvec[b]   = Wq @ h2_agg[b]                      (H)
    v[b,j,:]  = Wk[j*hd:(j+1)*hd, :]^T @ qvec[b, j*hd:(j+1)*hd]   (per head)
    raw[b,j,s] = v[b,j,:] . h1[b,s,:]
    w = softmax_s(scale*raw + key_mask);  row[b,s] = mean_j w[b,j,s]
    out[b,q,s] = row[b,s]

Each of the 8 cores runs the identical tiny compute and writes its own
[B, S/8, S] q-slice of the output; the host concatenates the slices.

h1, Wq, Wk are fed to the device as bf16 (f32 PSUM accumulation; output rel
err ~1e-3 vs the f32 reference), halving their DMA traffic; h1 and Wq are
staged pre-transposed so the PE contraction dim lands on SBUF partitions
with plain (full-bandwidth) DMA loads.  The 1/aspect_len factor is linear
through qvec/v/scores, so it is folded into the per-batch exp() scale
instead of scaling h2_agg up front.  Explicit scheduler deps keep the DMA
stream in consumption order: WqT (heads the PE chain), then Wk, then the 16
h1 tiles batch-0-first, so batch 0's softmax/stores overlap batch 1's loads.
"""

import os
from contextlib import ExitStack

import ml_dtypes
import numpy as np

import concourse.bass as bass
import concourse.tile as tile
from concourse import bacc
from concourse import mybir

B, S, A, H = 2, 2048, 16, 1024
NH, HD = 16, 64
SCALE = float(HD) ** -0.5
NCORES = 8
QS = S // NCORES  # q rows per core
NC_H = H // 128   # 8 contraction chunks
NEG = -1.0e30

F32 = mybir.dt.float32
F32R = mybir.dt.float32r
BF16 = mybir.dt.bfloat16
U8 = mybir.dt.uint8
AF = mybir.ActivationFunctionType


def _build_kernel(stage=99):
    nc = bacc.Bacc("TRN2")
    h1T_d = nc.dram_tensor("h1T", [B, H, S], BF16, kind="ExternalInput")
    h2 = nc.dram_tensor("h2", [B, A, H], F32, kind="ExternalInput")
    smask = nc.dram_tensor("smask", [B, S], U8, kind="ExternalInput")
    amask = nc.dram_tensor("amask", [B, A], U8, kind="ExternalInput")
    wqT_d = nc.dram_tensor("WqT", [H, H], BF16, kind="ExternalInput")
    wkb = nc.dram_tensor("Wkb", [H, H], BF16, kind="ExternalInput")
    if stage >= 99:
        out = nc.dram_tensor("out", [B, QS, S], F32, kind="ExternalOutput")
    elif stage == 2:
        out = nc.dram_tensor("out", [128, NC_H * B], F32, kind="ExternalOutput")
    elif stage == 3:
        out = nc.dram_tensor("out", [128, NC_H * B * NH], F32, kind="ExternalOutput")
    elif stage == 4:
        out = nc.dram_tensor("out", [B, NH, S], F32, kind="ExternalOutput")

    with tile.TileContext(nc) as tc, ExitStack() as ctx:
        consts = ctx.enter_context(tc.tile_pool(name="consts", bufs=1))
        small = ctx.enter_context(tc.tile_pool(name="small", bufs=2))
        wpool = ctx.enter_context(tc.tile_pool(name="wpool", bufs=3))
        wqp = ctx.enter_context(tc.tile_pool(name="wqp", bufs=8))
        wkp = ctx.enter_context(tc.tile_pool(name="wkp", bufs=8))
        h1tp = ctx.enter_context(tc.tile_pool(name="h1tp", bufs=16))
        big = ctx.enter_context(tc.tile_pool(name="big", bufs=2))
        pss = ctx.enter_context(tc.tile_pool(name="pss", bufs=1, space="PSUM"))
        psv = ctx.enter_context(tc.tile_pool(name="psv", bufs=1, space="PSUM"))
        psc = ctx.enter_context(tc.tile_pool(name="psc", bufs=2, space="PSUM"))
        psb = ctx.enter_context(tc.tile_pool(name="psb", bufs=1, space="PSUM"))

        ones128 = consts.tile([1, 128], F32, tag="ones128")
        nc.vector.memset(ones128, 1.0)
        ones16 = consts.tile([1, 16], BF16, tag="ones16")
        nc.vector.memset(ones16, 1.0)

        # ---- per-batch prep: aspect mask column, 1/len, key-mask row ----
        am_cols = []   # [A, 1] f32 per batch
        scl_t = []     # [16, 1] f32 exp scale = SCALE / aspect_len, per batch
        mb_t = []      # [1, S] bf16 additive key mask, per batch
        for b in range(B):
            am_row_u8 = small.tile([1, A], U8, tag="am_row_u8")
            nc.gpsimd.dma_start(am_row_u8, amask[b:b + 1, :])
            am_row = small.tile([1, A], F32, tag="am_row")
            nc.vector.tensor_copy(am_row, am_row_u8)
            alen = small.tile([1, 1], F32, tag="alen")
            nc.vector.reduce_sum(alen, am_row, axis=mybir.AxisListType.X)
            nc.vector.tensor_scalar_max(alen, alen, 1.0)
            rlen = small.tile([1, 1], F32, tag="rlen")
            nc.vector.reciprocal(rlen, alen)

            # [16, 1] mask column via PE transpose of the row (identity = 1.0)
            am_col_ps = pss.tile([A, 1], F32, tag="pssmall", name="am_col_ps")
            nc.tensor.transpose(am_col_ps, am_row, ones128[:, 0:1])
            am_col = small.tile([A, 1], F32, tag="am_col")
            nc.vector.tensor_copy(am_col, am_col_ps)
            am_cols.append(am_col)

            # broadcast rlen to 16 partitions, fold in softmax scale
            r16_ps = pss.tile([16, 1], F32, tag="pssmall", name="r16_ps")
            nc.tensor.matmul(r16_ps, lhsT=ones128[:, 0:16], rhs=rlen)
            scl = small.tile([16, 1], F32, tag="scl", name=f"scl{b}")
            nc.vector.tensor_scalar_mul(scl, r16_ps, SCALE)
            scl_t.append(scl)

            sm_u8 = small.tile([1, S], U8, tag="sm_u8")
            nc.gpsimd.dma_start(sm_u8, smask[b:b + 1, :])
            mb = small.tile([1, S], BF16, tag="mb")
            # mb = mask*1e30 - 1e30  -> 0 for valid, -1e30 for masked
            nc.scalar.activation(mb, sm_u8, AF.Copy, bias=NEG, scale=-NEG)
            mb_t.append(mb)

        # ---- all plain (non-transposed) big loads first: h2, Wk ----
        # (keeps the DMA stream in one XBAR mode; transposes follow as one
        # group, so only one passthrough->transpose transition happens)
        h2t_tiles = []
        plain_insts = []
        for b in range(B):
            h2t = small.tile([A, H], F32, tag="h2t", name=f"h2t{b}")
            plain_insts.append(nc.scalar.dma_start(h2t, h2[b]))
            h2t_tiles.append(h2t)
        # WqT first: it heads the PE critical chain (qv -> vt -> scores)
        wqT_tiles = []
        wq_insts = []
        from concourse.tile_rust import add_dep_helper
        for c in range(NC_H):
            wqT_c = wqp.tile([128, H], BF16, tag="wqT", name=f"wqT{c}")
            wq_insts.append(
                nc.sync.dma_start(wqT_c, wqT_d[c * 128:(c + 1) * 128, :]))
            wqT_tiles.append(wqT_c)
        for i in range(1, len(wq_insts)):
            add_dep_helper(wq_insts[i].ins, wq_insts[i - 1].ins,
                           sync=False, reason="wqT stream order")
        wk_tiles = []
        for c in range(NC_H):
            wk_c = wkp.tile([128, H], BF16, tag="wk", name=f"wk{c}")
            wk_i = nc.scalar.dma_start(wk_c, wkb[c * 128:(c + 1) * 128, :])
            add_dep_helper(wk_i.ins, wq_insts[-1].ins,
                           sync=False, reason="wk after wqT")
            wk_tiles.append(wk_c)
        h1t_tiles = {}
        h1_insts = []
        for b in range(B):
            for m in range(NC_H):
                h1t = h1tp.tile([128, S], BF16, tag="h1t", name=f"h1t_{b}_{m}")
                h1_insts.append(nc.sync.dma_start(
                    h1t, h1T_d[b, m * 128:(m + 1) * 128, :]))
                h1t_tiles[b, m] = h1t
        # stream h1 tiles in consumption order (b0 before b1), after wqT
        add_dep_helper(h1_insts[0].ins, wq_insts[-1].ins,
                       sync=False, reason="h1 after wqT")
        for i in range(1, len(h1_insts)):
            add_dep_helper(h1_insts[i].ins, h1_insts[i - 1].ins,
                           sync=False, reason="h1 stream order")

        # ---- h2sumT[i, (c, b)] = sum_a m[a] h2[b, a, i]  (unscaled) ----
        h2sT_ps = pss.tile([128, NC_H, B], F32, tag="pssmall", name="h2sT_ps")
        for b in range(B):
            for c in range(NC_H):
                nc.tensor.matmul(
                    h2sT_ps[:, c, b:b + 1],
                    lhsT=h2t_tiles[b][:, c * 128:(c + 1) * 128],
                    rhs=am_cols[b],
                )
        h2sT = small.tile([128, NC_H, B], BF16, tag="h2sT")
        nc.vector.tensor_copy(h2sT, h2sT_ps)

        # ---- qvec' = Wq @ h2sum (len factor folded into exp scale) ----
        # qv[o, (m, b)] accumulated over in-chunks c, via transposed Wq tiles
        qv_ps = pss.tile([128, NC_H, B], F32, tag="pssmall", name="qv_ps")
        for m in range(NC_H):
            for c in range(NC_H):
                nc.tensor.matmul(
                    qv_ps[:, m, :],
                    lhsT=wqT_tiles[c][:, m * 128:(m + 1) * 128],
                    rhs=h2sT[:, c, :],
                    start=(c == 0),
                    stop=(c == NC_H - 1),
                )
        qv = small.tile([128, NC_H, B], F32, tag="qv")
        nc.vector.tensor_copy(qv, qv_ps)

        if stage == 2:
            nc.scalar.dma_start(out[:, :], qv)

        # ---- vT[i, m-chunk, (j, b)]: o-chunk c covers heads {2c, 2c+1}
        # column index within a 32-block is j*2 + b = 4c + 2*jl + b
        vt_ps = psv.tile([128, NC_H, B * NH], F32, tag="psvt", name="vt_ps")
        for c in range(NC_H):
            # masked qvec columns (jl, b), head rows zeroed outside block
            qm = small.tile([128, 4], BF16, tag="qm")
            nc.vector.memset(qm, 0.0)
            for b in range(B):
                nc.vector.tensor_copy(qm[0:64, b:b + 1], qv[0:64, c, b:b + 1])
                nc.vector.tensor_copy(
                    qm[64:128, 2 + b:3 + b], qv[64:128, c, b:b + 1])
            for m in range(NC_H):
                nc.tensor.matmul(
                    vt_ps[:, m, 4 * c:4 * c + 4],
                    lhsT=wk_tiles[c][:, m * 128:(m + 1) * 128],
                    rhs=qm,
                )
        vt_bf = small.tile([128, NC_H, B * NH], BF16, tag="vt_bf")
        nc.vector.tensor_copy(vt_bf, vt_ps)
        # view with (j, b) split for per-batch weight slices
        vt4 = vt_bf.rearrange("p c (j b) -> p c j b", b=B)
        if stage == 3:
            vt_f32 = small.tile([128, NC_H * B * NH], F32, tag="vt_f32")
            nc.vector.tensor_copy(vt_f32, vt_ps)
            nc.scalar.dma_start(out[:, :], vt_f32)

        # ---- scores + softmax + broadcast + store, pipelined per batch ----
        HS = S // 2
        for b in range(B):
            sc_h = [
                psc.tile([16, HS], F32, tag="sc", name=f"sc_{b}_{h}")
                for h in range(2)
            ]
            for m in range(NC_H):
                h1t = h1t_tiles[b, m]
                for n in range(S // 512):
                    nc.tensor.matmul(
                        sc_h[n // 2][:, (n % 2) * 512:(n % 2 + 1) * 512],
                        lhsT=vt4[:, m, :, b],
                        rhs=h1t[:, n * 512:(n + 1) * 512],
                        start=(m == 0),
                        stop=False,
                    )
            for n in range(S // 512):
                nc.tensor.matmul(
                    sc_h[n // 2][:, (n % 2) * 512:(n % 2 + 1) * 512],
                    lhsT=ones16,
                    rhs=mb_t[b][:, n * 512:(n + 1) * 512],
                    start=False,
                    stop=True,
                )

            # w = exp(scale/len * scores), zsum = sum_s w (per half, summed)
            w_h = []
            zs_h = []
            for h in range(2):
                w_sb = big.tile([16, HS], F32R, tag="w_sb", name=f"w_{b}_{h}")
                zsum = small.tile([16, 1], F32, tag="zsum", name=f"z_{b}_{h}")
                nc.scalar.activation(
                    w_sb, sc_h[h], AF.Exp,
                    bias=0.0, scale=scl_t[b], accum_out=zsum,
                )
                w_h.append(w_sb)
                zs_h.append(zsum)
            if stage == 4:
                for h in range(2):
                    w_f32 = big.tile([16, HS], F32, tag="w_f32")
                    nc.vector.tensor_copy(w_f32, w_h[h])
                    nc.scalar.dma_start(out[b, :, h * HS:(h + 1) * HS], w_f32)
                continue
            ztot = small.tile([16, 1], F32, tag="ztot", name=f"zt_{b}")
            nc.vector.tensor_add(ztot, zs_h[0], zs_h[1])
            nc.vector.tensor_scalar_mul(ztot, ztot, float(NH))
            rz = small.tile([16, 1], F32, tag="rz")
            nc.vector.reciprocal(rz, ztot)
            ones_l = small.tile([16, 128], F32, tag="ones_l")
            nc.vector.memset(ones_l, 1.0)
            lmat = small.tile([16, 128], F32R, tag="lmat")
            nc.vector.tensor_scalar_mul(lmat, ones_l, rz)

            # out rows: bc[q, s] = sum_j lmat[j, q] * w[j, s], in column halves
            for h in range(2):
                bc = psb.tile([128, HS], F32, tag="bc", name="bc")
                for n in range(2):
                    nc.tensor.matmul(
                        bc[:, n * 512:(n + 1) * 512],
                        lhsT=lmat,
                        rhs=w_h[h][:, n * 512:(n + 1) * 512],
                    )
                obuf = big.tile([128, HS], F32, tag="obuf")
                nc.vector.tensor_copy(obuf, bc)
                if stage >= 99:
                    rep = bass.AP(
                        tensor=obuf.tensor, offset=obuf.offset,
                        ap=[list(obuf.ap[0]), [0, QS // 128],
                            list(obuf.ap[1])])
                    nc.scalar.dma_start(
                        out[b, :, h * HS:(h + 1) * HS].rearrange(
                            "(t p) c -> p t c", p=128),
                        rep,
                    )

    nc.finalize()
    return nc


_NC_CACHE = None


def kernel(h1, h2, sentence_mask, aspect_mask, Wq, Wk):
    global _NC_CACHE
    from concourse.bass_utils import run_bass_kernel_spmd

    if _NC_CACHE is None:
        _NC_CACHE = _build_kernel()
    nc = _NC_CACHE

    in_map = {
        "h1T": np.ascontiguousarray(
            np.asarray(h1).astype(ml_dtypes.bfloat16).transpose(0, 2, 1)),
        "h2": np.ascontiguousarray(h2, dtype=np.float32),
        "smask": np.ascontiguousarray(sentence_mask).view(np.uint8),
        "amask": np.ascontiguousarray(aspect_mask).view(np.uint8),
        "WqT": np.ascontiguousarray(
            np.asarray(Wq).astype(ml_dtypes.bfloat16).T),
        "Wkb": np.ascontiguousarray(Wk).astype(ml_dtypes.bfloat16),
    }
    trace = bool(int(os.environ.get("KERNEL_TRACE", "0")))
    res = run_bass_kernel_spmd(
        nc,
        [dict(in_map) for _ in range(NCORES)],
        core_ids=list(range(NCORES)),
        trace=trace,
    )
    if trace and res.exec_time_ns is not None:
        kernel.last_exec_time_ns = res.exec_time_ns
        kernel.last_results = res
    return np.concatenate([r["out"] for r in res.results], axis=1)



# revision 30
# speedup vs baseline: 1.5984x; 1.5984x over previous
"""Trainium2 Bass kernel for nn_CrossAttentionModule.

Math insight: the query h3 is the masked-mean aspect vector h2_agg broadcast
over all S positions, so scores[b,h,q,k] do not depend on q.  The whole
[B,S,S] output is a single row row[b,k] broadcast along the q axis:

    qvec[b]   = Wq @ h2_agg[b]                      (H)
    v[b,j,:]  = Wk[j*hd:(j+1)*hd, :]^T @ qvec[b, j*hd:(j+1)*hd]   (per head)
    raw[b,j,s] = v[b,j,:] . h1[b,s,:]
    w = softmax_s(scale*raw + key_mask);  row[b,s] = mean_j w[b,j,s]
    out[b,q,s] = row[b,s]

Each of the 8 cores runs the identical tiny compute and writes its own
[B, S/8, S] q-slice of the output; the host concatenates the slices.

h1, Wq, Wk are fed to the device as fp8 e3m4 with power-of-two scales
(h1*2, W*128; f32 PSUM accumulation; output rel err ~6e-3 vs the f32
reference).  Device intermediates (h2sum, qvec, v) are requantized to
e3m4 with power-of-two rescales chosen so the net factor through the
score matmul is exactly 1.0 — the per-batch exp() scale only carries
SCALE/aspect_len as in f32.

The kernel is DMA-bound (~29.3us of serial DMA-pool transfers at 360
GB/s), so everything is organized to keep the pool streaming:
  - few big DMA instructions (per-DMA fixed costs ~1.2us): one for Wq,
    one for Wk, two column-halves per batch for h1, one store per batch;
    masks+h2 go through SWDGE (gpsimd) to keep the HWDGE head free.
  - stream order WqT -> Wk -> h1(b0) -> h1(b1) -> store(b0) -> store(b1).
  - softmax is pipelined in 512-column chunks (scores -> +mask -> exp)
    so only the last chunk's work trails the final h1 piece.
  - junk matmuls keep the PE continuously busy from t~0.7us so it is at
    the full 2.4 GHz p-state when the real score matmuls arrive.
"""

import os
from contextlib import ExitStack

import ml_dtypes
import numpy as np

import concourse.bass as bass
import concourse.tile as tile
from concourse import bacc
from concourse import mybir

B, S, A, H = 2, 2048, 16, 1024
NH, HD = 16, 64
SCALE = float(HD) ** -0.5
NCORES = 8
QS = S // NCORES  # q rows per core
NC_H = H // 128   # 8 contraction chunks
NEG = -1.0e30
NCK = 4           # 512-column softmax chunks
CK = S // NCK

F32 = mybir.dt.float32
F32R = mybir.dt.float32r
BF16 = mybir.dt.bfloat16
F8 = mybir.dt.float8e3
F8E4 = mybir.dt.float8e4
U8 = mybir.dt.uint8
AF = mybir.ActivationFunctionType
DR = mybir.MatmulPerfMode.DoubleRow

# power-of-two quantization scales (see module docstring)
S_H1 = 2.0       # host: h1 * S_H1 -> e3m4
S_W = 128.0      # host: Wq*S_W, Wk*S_W -> e3m4
S_H2S = 0.125    # device: h2sum * S_H2S -> e3m4
S_QM = 0.5       # device: qm = qv_true * S_QM
S_VT = 0.5       # device: vt = v_true * S_VT  (S_VT * S_H1 == 1 -> scl unchanged)


def _build_kernel(warm=(6, 14, 14, 12)):
    nc = bacc.Bacc("TRN2")
    h1T_d = nc.dram_tensor("h1T", [B, H, S], F8E4, kind="ExternalInput")
    h2 = nc.dram_tensor("h2", [B, A, H], BF16, kind="ExternalInput")
    masks_d = nc.dram_tensor("masks", [1, B, S + A], U8, kind="ExternalInput")
    wqT_d = nc.dram_tensor("WqT", [H, H], F8, kind="ExternalInput")
    wkb = nc.dram_tensor("Wkb", [H, H], F8, kind="ExternalInput")
    out = nc.dram_tensor("out", [B, QS, S], F32, kind="ExternalOutput")

    from concourse.tile_rust import add_dep_helper

    with tile.TileContext(nc) as tc, ExitStack() as ctx:
        consts = ctx.enter_context(tc.tile_pool(name="consts", bufs=1))
        small = ctx.enter_context(tc.tile_pool(name="small", bufs=2))
        wqp = ctx.enter_context(tc.tile_pool(name="wqp", bufs=1))
        wkp = ctx.enter_context(tc.tile_pool(name="wkp", bufs=1))
        h1tp = ctx.enter_context(tc.tile_pool(name="h1tp", bufs=4))
        wpool = ctx.enter_context(tc.tile_pool(name="wpool", bufs=4))
        obp = ctx.enter_context(tc.tile_pool(name="obp", bufs=2))
        pss = ctx.enter_context(tc.tile_pool(name="pss", bufs=1, space="PSUM"))
        psv = ctx.enter_context(tc.tile_pool(name="psv", bufs=1, space="PSUM"))
        psc = ctx.enter_context(tc.tile_pool(name="psc", bufs=2, space="PSUM"))
        psb = ctx.enter_context(tc.tile_pool(name="psb", bufs=4, space="PSUM"))

        ones128 = consts.tile([1, 128], F32, tag="ones128")
        nc.vector.memset(ones128, 1.0)
        ones16 = consts.tile([1, 16], BF16, tag="ones16")
        nc.vector.memset(ones16, 1.0)
        junk = consts.tile([128, 512], BF16, tag="junk")
        nc.vector.memset(junk, 0.0)

        def pe_warm(n, name):
            for i in range(n):
                jp = psb.tile([128, 512], F32, tag="bc", name=f"{name}{i}")
                nc.tensor.matmul(jp, lhsT=junk[:, 0:128], rhs=junk)

        # Exp act-table preload, long before the first real exp
        dume = small.tile([1, 16], F32, tag="dume")
        nc.scalar.activation(dume, ones128[:, 0:16], AF.Exp)

        # ---- the DMA stream: WqT, masks, h2, Wk, h1 column-halves (b0
        # first); stores ride the scalar queue at the end.
        wqT = wqp.tile([128, NC_H, H], F8, tag="wqT")
        i_wq = nc.sync.dma_start(
            wqT, wqT_d.rearrange("(c p) h -> p c h", p=128))
        mask_sb = small.tile([1, B, S + A], U8, tag="mask_sb")
        i_mask = nc.sync.dma_start(mask_sb, masks_d[:, :, :])
        h2t = small.tile([A, B, H], BF16, tag="h2t")
        i_h2 = nc.sync.dma_start(h2t, h2.rearrange("b a h -> a b h"))
        wk = wkp.tile([128, NC_H, H], F8, tag="wk")
        i_wk = nc.sync.dma_start(
            wk, wkb.rearrange("(c p) h -> p c h", p=128))
        h1t = {}
        h1_insts = []
        for b in range(B):
            for piece in range(2):
                t = h1tp.tile([128, NC_H, S // 2], F8E4, tag="h1t",
                              name=f"h1t_{b}_{piece}")
                lo = piece * (S // 2)
                h1_insts.append(nc.sync.dma_start(
                    t, h1T_d[b, :, lo:lo + S // 2].rearrange(
                        "(c p) s -> p c s", p=128)))
                h1t[b, piece] = t
        chain = [i_wq, i_mask, i_h2, i_wk] + h1_insts
        for i in range(1, len(chain)):
            add_dep_helper(chain[i].ins, chain[i - 1].ins,
                           sync=False, reason="dma stream order")

        pe_warm(warm[0], "w0_")

        # ---- per-batch prep: aspect mask column, 1/len, key-mask row ----
        am_cols = []   # [A, 1] bf16 per batch
        scl_t = []     # [16, 1] f32 exp scale = SCALE / aspect_len, per batch
        mb_t = []      # [1, S] bf16 additive key mask, per batch
        for b in range(B):
            am_row = small.tile([1, A], F32, tag="am_row")
            nc.vector.tensor_copy(am_row, mask_sb[0:1, b, S:S + A])
            alen = small.tile([1, 1], F32, tag="alen")
            nc.vector.reduce_sum(alen, am_row, axis=mybir.AxisListType.X)
            nc.vector.tensor_scalar_max(alen, alen, 1.0)
            rlen = small.tile([1, 1], F32, tag="rlen")
            nc.vector.reciprocal(rlen, alen)

            # [16, 1] mask column via PE transpose of the row (identity = 1.0)
            am_col_ps = pss.tile([A, 1], F32, tag="pssmall", name="am_col_ps")
            nc.tensor.transpose(am_col_ps, am_row, ones128[:, 0:1])
            am_col = small.tile([A, 1], BF16, tag="am_col")
            nc.vector.tensor_copy(am_col, am_col_ps)
            am_cols.append(am_col)

            # broadcast rlen to 16 partitions, fold in softmax scale
            r16_ps = pss.tile([16, 1], F32, tag="pssmall", name="r16_ps")
            nc.tensor.matmul(r16_ps, lhsT=ones128[:, 0:16], rhs=rlen)
            scl = small.tile([16, 1], F32, tag="scl", name=f"scl{b}")
            nc.vector.tensor_scalar_mul(scl, r16_ps, SCALE)
            scl_t.append(scl)

            mb = small.tile([1, S], BF16, tag="mb")
            # mb = mask*1e30 - 1e30  -> 0 for valid, -1e30 for masked
            nc.scalar.activation(mb, mask_sb[0:1, b, 0:S], AF.Copy,
                                 bias=NEG, scale=-NEG)
            mb_t.append(mb)

        # ---- h2sumT[i, (c, b)] = sum_a m[a] h2[b, a, i]  (unscaled) ----
        h2sT_ps = pss.tile([128, NC_H, B], F32, tag="pssmall", name="h2sT_ps")
        for b in range(B):
            for c in range(NC_H):
                nc.tensor.matmul(
                    h2sT_ps[:, c, b:b + 1],
                    lhsT=h2t[:, b, c * 128:(c + 1) * 128],
                    rhs=am_cols[b],
                )
        h2sT = small.tile([128, NC_H, B], F8, tag="h2sT")
        nc.vector.tensor_scalar_mul(h2sT, h2sT_ps, S_H2S)

        pe_warm(warm[1], "w1_")

        # ---- qvec' = Wq @ h2sum (len factor folded into exp scale) ----
        # qv[o, (m, b)] accumulated over in-chunks c, via transposed Wq tiles
        qv_ps = pss.tile([128, NC_H, B], F32, tag="pssmall", name="qv_ps")
        for m in range(NC_H):
            for c in range(NC_H):
                nc.tensor.matmul(
                    qv_ps[:, m, :],
                    lhsT=wqT[:, c, m * 128:(m + 1) * 128],
                    rhs=h2sT[:, c, :],
                    start=(c == 0),
                    stop=(c == NC_H - 1),
                )
        qv = small.tile([128, NC_H, B], F32, tag="qv")
        nc.vector.tensor_copy(qv, qv_ps)

        pe_warm(warm[2], "w2_")

        # ---- vT[i, m-chunk, (j, b)]: o-chunk c covers heads {2c, 2c+1}
        # column index within a 32-block is j*2 + b = 4c + 2*jl + b
        vt_ps = psv.tile([128, NC_H, B * NH], F32, tag="psvt", name="vt_ps")
        qm_scale = S_QM / (S_W * S_H2S)
        # masked qvec columns (jl, b) for every chunk c in one strided op
        # each: head rows zeroed outside their 64-row block by the memset
        qm = small.tile([128, NC_H, 4], F8, tag="qm")
        nc.vector.memset(qm, 0.0)
        nc.vector.tensor_scalar_mul(
            qm[0:64, :, 0:2], qv[0:64, :, :], qm_scale)
        nc.vector.tensor_scalar_mul(
            qm[64:128, :, 2:4], qv[64:128, :, :], qm_scale)
        for c in range(NC_H):
            for m in range(NC_H):
                nc.tensor.matmul(
                    vt_ps[:, m, 4 * c:4 * c + 4],
                    lhsT=wk[:, c, m * 128:(m + 1) * 128],
                    rhs=qm[:, c, :],
                )
        vt_f8 = small.tile([128, NC_H, B * NH], F8E4, tag="vt_f8")
        nc.vector.tensor_scalar_mul(vt_f8, vt_ps, S_VT / (S_W * S_QM))
        # view with (j, b) split for per-batch weight slices
        vt4 = vt_f8.rearrange("p c (j b) -> p c j b", b=B)

        pe_warm(warm[3], "w3_")

        # ---- scores + softmax in 512-col chunks + broadcast + store ----
        ones_l = consts.tile([16, 128], F32, tag="ones_l")
        nc.vector.memset(ones_l, 1.0)
        for b in range(B):
            w_n = []
            zbuf = small.tile([16, NCK], F32, tag="zbuf", name=f"zbuf_{b}")
            for n in range(NCK):
                piece, col = n // 2, (n % 2) * CK
                sc = psc.tile([16, CK], F32, tag="sc", name=f"sc_{b}_{n}")
                for m2 in range(NC_H // 2):
                    # DoubleRow: two 128-deep k-tiles per instruction
                    nc.tensor.matmul(
                        sc,
                        lhsT=vt4[:, 2 * m2:2 * m2 + 2, :, b],
                        rhs=h1t[b, piece][:, 2 * m2:2 * m2 + 2, col:col + CK],
                        start=(m2 == 0),
                        stop=False,
                        perf_mode=DR,
                    )
                nc.tensor.matmul(
                    sc, lhsT=ones16,
                    rhs=mb_t[b][:, n * CK:(n + 1) * CK],
                    start=False, stop=True,
                )
                # w = exp(scale/len * scores), zsum = sum_cols w
                w_sb = wpool.tile([16, CK], F32R, tag="w", name=f"w_{b}_{n}")
                nc.scalar.activation(
                    w_sb, sc, AF.Exp, bias=0.0, scale=scl_t[b],
                    accum_out=zbuf[:, n:n + 1],
                )
                w_n.append(w_sb)

            # per-head 1/(16*Z_j) row weights for the broadcast matmul
            ztot = small.tile([16, 1], F32, tag="ztot", name=f"zt_{b}")
            nc.vector.reduce_sum(ztot, zbuf, axis=mybir.AxisListType.X)
            nc.vector.tensor_scalar_mul(ztot, ztot, float(NH))
            rz = small.tile([16, 1], F32, tag="rz")
            nc.vector.reciprocal(rz, ztot)
            lmat = small.tile([16, 128], F32R, tag="lmat")
            nc.vector.tensor_scalar_mul(lmat, ones_l, rz)

            # out rows: bc[q, s] = sum_j lmat[j, q] * w[j, s], per chunk;
            # first two chunk copies ride Act, last two DVE (parallel tail)
            obuf = obp.tile([128, S], F32, tag="obuf", name=f"obuf{b}")
            for n in range(NCK):
                bc = psb.tile([128, CK], F32, tag="bc", name=f"bc_{b}_{n}")
                nc.tensor.matmul(bc, lhsT=lmat, rhs=w_n[n])
                if n < 2:
                    nc.scalar.copy(obuf[:, n * CK:(n + 1) * CK], bc)
                else:
                    nc.vector.tensor_copy(obuf[:, n * CK:(n + 1) * CK], bc)
            rep = bass.AP(
                tensor=obuf.tensor, offset=obuf.offset,
                ap=[list(obuf.ap[0]), [0, QS // 128], list(obuf.ap[1])])
            nc.scalar.dma_start(
                out[b].rearrange("(t p) c -> p t c", p=128), rep)

    nc.finalize()
    return nc


_NC_CACHE = None


def kernel(h1, h2, sentence_mask, aspect_mask, Wq, Wk):
    global _NC_CACHE
    from concourse.bass_utils import run_bass_kernel_spmd

    if _NC_CACHE is None:
        _NC_CACHE = _build_kernel()
    nc = _NC_CACHE

    f8 = ml_dtypes.float8_e3m4
    in_map = {
        "h1T": np.ascontiguousarray(
            np.clip(np.asarray(h1, np.float32) * S_H1, -240.0, 240.0)
            .astype(ml_dtypes.float8_e4m3).transpose(0, 2, 1)),
        "h2": np.ascontiguousarray(np.asarray(h2)).astype(ml_dtypes.bfloat16),
        "masks": np.ascontiguousarray(np.concatenate(
            [np.asarray(sentence_mask), np.asarray(aspect_mask)],
            axis=1)).view(np.uint8).reshape(1, B, S + A),
        "WqT": np.ascontiguousarray(
            np.clip(np.asarray(Wq, np.float32) * S_W, -15.5, 15.5)
            .astype(f8).T),
        "Wkb": np.clip(np.asarray(Wk, np.float32) * S_W, -15.5, 15.5)
        .astype(f8),
    }
    trace = bool(int(os.environ.get("KERNEL_TRACE", "0")))
    res = run_bass_kernel_spmd(
        nc,
        [dict(in_map) for _ in range(NCORES)],
        core_ids=list(range(NCORES)),
        trace=trace,
    )
    if trace and res.exec_time_ns is not None:
        kernel.last_exec_time_ns = res.exec_time_ns
        kernel.last_results = res
    return np.concatenate([r["out"] for r in res.results], axis=1)


# revision 50
# speedup vs baseline: 1.8361x; 1.1487x over previous
"""Trainium2 Bass kernel for nn_CrossAttentionModule.

Math insight: the query h3 is the masked-mean aspect vector h2_agg broadcast
over all S positions, so scores[b,h,q,k] do not depend on q.  The whole
[B,S,S] output is a single row row[b,k] broadcast along the q axis:

    qvec[b]   = Wq @ h2_agg[b]                      (H)
    v[b,j,:]  = Wk[j*hd:(j+1)*hd, :]^T @ qvec[b, j*hd:(j+1)*hd]   (per head)
    raw[b,j,s] = v[b,j,:] . h1[b,s,:]
    w = softmax_s(scale*raw + key_mask);  row[b,s] = mean_j w[b,j,s]
    out[b,q,s] = row[b,s]

Each of the 8 cores runs the identical tiny compute and writes its own
[B, S/8, S] q-slice of the output; the host concatenates the slices.

h1, Wq, Wk are fed to the device as fp8 e3m4 with power-of-two scales
(h1*2, W*128; f32 PSUM accumulation; output rel err ~6e-3 vs the f32
reference).  Device intermediates (h2sum, qvec, v) are requantized to
e3m4 with power-of-two rescales chosen so the net factor through the
score matmul is exactly 1.0 — the per-batch exp() scale only carries
SCALE/aspect_len as in f32.

The kernel is DMA-bound (~29.3us of serial DMA-pool transfers at 360
GB/s), so everything is organized to keep the pool streaming:
  - few big DMA instructions (per-DMA fixed costs ~1.2us): one for Wq,
    one for Wk, two column-halves per batch for h1, one store per batch;
    masks+h2 go through SWDGE (gpsimd) to keep the HWDGE head free.
  - stream order WqT -> Wk -> h1(b0) -> h1(b1) -> store(b0) -> store(b1).
  - softmax is pipelined in 512-column chunks (scores -> +mask -> exp)
    so only the last chunk's work trails the final h1 piece.
  - junk matmuls keep the PE continuously busy from t~0.7us so it is at
    the full 2.4 GHz p-state when the real score matmuls arrive.
"""

import os
from contextlib import ExitStack

import ml_dtypes
import numpy as np

import concourse.bass as bass
import concourse.tile as tile
from concourse import bacc
from concourse import mybir

B, S, A, H = 2, 2048, 16, 1024
NH, HD = 16, 64
SCALE = float(HD) ** -0.5
NCORES = 8
QS = S // NCORES  # q rows per core
NC_H = H // 128   # 8 contraction chunks
NEG = -1.0e30
NCK = 4           # 512-column softmax chunks
CK = S // NCK

F32 = mybir.dt.float32
F32R = mybir.dt.float32r
F16 = mybir.dt.float16
BF16 = mybir.dt.bfloat16
F8 = mybir.dt.float8e3
F8E4 = mybir.dt.float8e4
U8 = mybir.dt.uint8
AF = mybir.ActivationFunctionType
DR = mybir.MatmulPerfMode.DoubleRow

# power-of-two quantization scales (see module docstring)
S_H1 = 2.0       # host: h1 * S_H1 -> e3m4
S_W = 128.0      # host: Wq*S_W, Wk*S_W -> e3m4
S_H2S = 0.125    # device: h2sum * S_H2S -> e3m4
S_QM = 0.5       # device: qm = qv_true * S_QM
S_VT = 0.5       # device: vt = v_true * S_VT  (S_VT * S_H1 == 1 -> scl unchanged)


def _build_kernel(warm=(6, 14, 14, 12)):
    nc = bacc.Bacc("TRN2")
    h1T_d = nc.dram_tensor("h1T", [B, H, S], F8E4, kind="ExternalInput")
    h2 = nc.dram_tensor("h2", [B, A, H], BF16, kind="ExternalInput")
    masks_d = nc.dram_tensor("masks", [1, B, S + A], U8, kind="ExternalInput")
    wqT_d = nc.dram_tensor("WqT", [H, H], F8, kind="ExternalInput")
    wkb = nc.dram_tensor("Wkb", [H, H], F8, kind="ExternalInput")
    out = nc.dram_tensor("out", [B, QS, S], F16, kind="ExternalOutput")

    from concourse.tile_rust import add_dep_helper

    with tile.TileContext(nc) as tc, ExitStack() as ctx:
        consts = ctx.enter_context(tc.tile_pool(name="consts", bufs=1))
        small = ctx.enter_context(tc.tile_pool(name="small", bufs=2))
        wqp = ctx.enter_context(tc.tile_pool(name="wqp", bufs=1))
        wkp = ctx.enter_context(tc.tile_pool(name="wkp", bufs=1))
        h1tp = ctx.enter_context(tc.tile_pool(name="h1tp", bufs=6))
        wpool = ctx.enter_context(tc.tile_pool(name="wpool", bufs=8))
        obp = ctx.enter_context(tc.tile_pool(name="obp", bufs=2))
        pss = ctx.enter_context(tc.tile_pool(name="pss", bufs=1, space="PSUM"))
        psv = ctx.enter_context(tc.tile_pool(name="psv", bufs=1, space="PSUM"))
        psc = ctx.enter_context(tc.tile_pool(name="psc", bufs=2, space="PSUM"))
        psb = ctx.enter_context(tc.tile_pool(name="psb", bufs=4, space="PSUM"))

        ones128 = consts.tile([1, 128], F32, tag="ones128")
        nc.vector.memset(ones128, 1.0)
        ones16 = consts.tile([1, 16], BF16, tag="ones16")
        nc.vector.memset(ones16, 1.0)
        junk = consts.tile([128, 512], BF16, tag="junk")
        nc.vector.memset(junk, 0.0)

        def pe_warm(n, name):
            for i in range(n):
                jp = psb.tile([128, 512], F32, tag="bc", name=f"{name}{i}")
                nc.tensor.matmul(jp, lhsT=junk[:, 0:128], rhs=junk)

        # Exp act-table preload, long before the first real exp
        dume = small.tile([1, 16], F32, tag="dume")
        nc.scalar.activation(dume, ones128[:, 0:16], AF.Exp)

        # ---- the DMA stream: WqT, masks, h2, Wk, h1 column-halves (b0
        # first); stores ride the scalar queue at the end.
        wqT = wqp.tile([128, NC_H, H], F8, tag="wqT")
        i_wq = nc.sync.dma_start(
            wqT, wqT_d.rearrange("(c p) h -> p c h", p=128))
        mask_sb = small.tile([1, B, S + A], U8, tag="mask_sb")
        i_mask = nc.sync.dma_start(mask_sb, masks_d[:, :, :])
        h2t = small.tile([A, B, H], BF16, tag="h2t")
        i_h2 = nc.sync.dma_start(h2t, h2.rearrange("b a h -> a b h"))
        wk = wkp.tile([128, NC_H, H], F8, tag="wk")
        i_wk = nc.sync.dma_start(
            wk, wkb.rearrange("(c p) h -> p c h", p=128))
        # h1 column pieces [1024, 512, 512] per batch: the two trailing
        # pieces keep only one softmax chunk on the post-load tail each
        PIECES = (1024, 512, 512)
        h1t = {}
        h1_insts = []
        for b in range(B):
            lo = 0
            for piece, pw in enumerate(PIECES):
                t = h1tp.tile([128, NC_H, pw], F8E4, tag=f"h1t{piece}",
                              name=f"h1t_{b}_{piece}")
                h1_insts.append(nc.sync.dma_start(
                    t, h1T_d[b, :, lo:lo + pw].rearrange(
                        "(c p) s -> p c s", p=128)))
                h1t[b, piece] = t
                lo += pw
        chain = [i_wq, i_mask, i_h2, i_wk] + h1_insts
        for i in range(1, len(chain)):
            add_dep_helper(chain[i].ins, chain[i - 1].ins,
                           sync=False, reason="dma stream order")

        pe_warm(warm[0], "w0_")

        # ---- per-batch prep: aspect mask column, 1/len, key-mask row ----
        am_cols = []   # [A, 1] bf16 per batch
        scl_t = []     # [16, 1] f32 exp scale = SCALE / aspect_len, per batch
        mb_t = []      # [1, S] bf16 additive key mask, per batch
        for b in range(B):
            am_row = small.tile([1, A], F32, tag="am_row")
            nc.vector.tensor_copy(am_row, mask_sb[0:1, b, S:S + A])
            alen = small.tile([1, 1], F32, tag="alen")
            nc.vector.reduce_sum(alen, am_row, axis=mybir.AxisListType.X)
            nc.vector.tensor_scalar_max(alen, alen, 1.0)
            rlen = small.tile([1, 1], F32, tag="rlen")
            nc.vector.reciprocal(rlen, alen)

            # [16, 1] mask column via PE transpose of the row (identity = 1.0)
            am_col_ps = pss.tile([A, 1], F32, tag="pssmall", name="am_col_ps")
            nc.tensor.transpose(am_col_ps, am_row, ones128[:, 0:1])
            am_col = small.tile([A, 1], BF16, tag="am_col")
            nc.vector.tensor_copy(am_col, am_col_ps)
            am_cols.append(am_col)

            # broadcast rlen to 16 partitions, fold in softmax scale
            r16_ps = pss.tile([16, 1], F32, tag="pssmall", name="r16_ps")
            nc.tensor.matmul(r16_ps, lhsT=ones128[:, 0:16], rhs=rlen)
            scl = small.tile([16, 1], F32, tag="scl", name=f"scl{b}")
            nc.vector.tensor_scalar_mul(scl, r16_ps, SCALE)
            scl_t.append(scl)

            mb = small.tile([1, S], BF16, tag="mb")
            # mb = mask*1e30 - 1e30  -> 0 for valid, -1e30 for masked
            nc.scalar.activation(mb, mask_sb[0:1, b, 0:S], AF.Copy,
                                 bias=NEG, scale=-NEG)
            mb_t.append(mb)

        # ---- h2sumT[i, (c, b)] = sum_a m[a] h2[b, a, i]  (unscaled) ----
        h2sT_ps = pss.tile([128, NC_H, B], F32, tag="pssmall", name="h2sT_ps")
        for b in range(B):
            for c in range(NC_H):
                nc.tensor.matmul(
                    h2sT_ps[:, c, b:b + 1],
                    lhsT=h2t[:, b, c * 128:(c + 1) * 128],
                    rhs=am_cols[b],
                )
        h2sT = small.tile([128, NC_H, B], F8, tag="h2sT")
        nc.vector.tensor_scalar_mul(h2sT, h2sT_ps, S_H2S)

        pe_warm(warm[1], "w1_")

        # ---- qvec' = Wq @ h2sum (len factor folded into exp scale) ----
        # qv[o, (m, b)] accumulated over in-chunks c, via transposed Wq tiles
        qv_ps = pss.tile([128, NC_H, B], F32, tag="pssmall", name="qv_ps")
        for m in range(NC_H):
            for c in range(NC_H):
                nc.tensor.matmul(
                    qv_ps[:, m, :],
                    lhsT=wqT[:, c, m * 128:(m + 1) * 128],
                    rhs=h2sT[:, c, :],
                    start=(c == 0),
                    stop=(c == NC_H - 1),
                )
        qv = small.tile([128, NC_H, B], F32, tag="qv")
        nc.vector.tensor_copy(qv, qv_ps)

        pe_warm(warm[2], "w2_")

        # ---- vT[i, m-chunk, (j, b)]: o-chunk c covers heads {2c, 2c+1}
        # column index within a 32-block is j*2 + b = 4c + 2*jl + b
        vt_ps = psv.tile([128, NC_H, B * NH], F32, tag="psvt", name="vt_ps")
        qm_scale = S_QM / (S_W * S_H2S)
        # masked qvec columns (jl, b) for every chunk c in one strided op
        # each: head rows zeroed outside their 64-row block by the memset
        qm = small.tile([128, NC_H, 4], F8, tag="qm")
        nc.vector.memset(qm, 0.0)
        nc.vector.tensor_scalar_mul(
            qm[0:64, :, 0:2], qv[0:64, :, :], qm_scale)
        nc.vector.tensor_scalar_mul(
            qm[64:128, :, 2:4], qv[64:128, :, :], qm_scale)
        for c in range(NC_H):
            for m in range(NC_H):
                nc.tensor.matmul(
                    vt_ps[:, m, 4 * c:4 * c + 4],
                    lhsT=wk[:, c, m * 128:(m + 1) * 128],
                    rhs=qm[:, c, :],
                )
        vt_f8 = small.tile([128, NC_H, B * NH], F8E4, tag="vt_f8")
        nc.vector.tensor_scalar_mul(vt_f8, vt_ps, S_VT / (S_W * S_QM))
        # view with (j, b) split for per-batch weight slices
        vt4 = vt_f8.rearrange("p c (j b) -> p c j b", b=B)

        pe_warm(warm[3], "w3_")

        # ---- scores + softmax in 512-col chunks, both batches ----
        # ones_l carries the 1/NH head-mean factor so lmat = 1/(NH * Z_j)
        ones_l = consts.tile([16, 128], F16, tag="ones_l")
        nc.vector.memset(ones_l, 1.0 / NH)
        w_all = {}
        zbufs = []
        for b in range(B):
            zbuf = small.tile([16, NCK], F32, tag="zbuf", name=f"zbuf_{b}")
            zbufs.append(zbuf)
            for n in range(NCK):
                piece, col = (0, n * CK) if n < 2 else (n - 1, 0)
                sc = psc.tile([16, CK], F32, tag="sc", name=f"sc_{b}_{n}")
                for m2 in range(NC_H // 2):
                    # DoubleRow: two 128-deep k-tiles per instruction
                    nc.tensor.matmul(
                        sc,
                        lhsT=vt4[:, 2 * m2:2 * m2 + 2, :, b],
                        rhs=h1t[b, piece][:, 2 * m2:2 * m2 + 2, col:col + CK],
                        start=(m2 == 0),
                        stop=False,
                        perf_mode=DR,
                    )
                nc.tensor.matmul(
                    sc, lhsT=ones16,
                    rhs=mb_t[b][:, n * CK:(n + 1) * CK],
                    start=False, stop=True,
                )
                # w = exp(scale/len * scores), zsum = sum_cols w
                w_sb = wpool.tile([16, CK], F16, tag="w", name=f"w_{b}_{n}")
                nc.scalar.activation(
                    w_sb, sc, AF.Exp, bias=0.0, scale=scl_t[b],
                    accum_out=zbuf[:, n:n + 1])
                w_all[b, n] = w_sb

        # ---- normalizer, head-mean broadcast, store (per batch) ----
        for b in range(B):
            ztot = small.tile([16, 1], F32, tag="ztot", name=f"zt_{b}")
            nc.vector.reduce_sum(ztot, zbufs[b], axis=mybir.AxisListType.X)
            rz = small.tile([16, 1], F32, tag="rz")
            nc.vector.reciprocal(rz, ztot)
            lmat = small.tile([16, 128], F16, tag="lmat")
            nc.vector.tensor_scalar_mul(lmat, ones_l, rz)

            # out rows: bc[q, s] = sum_j lmat[j, q] * w[j, s], per chunk;
            # first two chunk copies ride DVE (starts immediately), last two
            # Act (free once the exps drain); store per column-half so the
            # first half's store issue overlaps the second half's copies
            obuf = obp.tile([128, S], F16, tag="obuf", name=f"obuf{b}")
            for n in range(NCK):
                bc = psb.tile([128, CK], F32, tag="bc", name=f"bc_{b}_{n}")
                nc.tensor.matmul(bc, lhsT=lmat, rhs=w_all[b, n])
                if n % 2 == 0:
                    nc.vector.tensor_copy(obuf[:, n * CK:(n + 1) * CK], bc)
                else:
                    nc.scalar.copy(obuf[:, n * CK:(n + 1) * CK], bc)
                if n % 2 == 1:
                    h = obuf[:, (n - 1) * CK:(n + 1) * CK]
                    rep = bass.AP(
                        tensor=h.tensor, offset=h.offset,
                        ap=[list(h.ap[0]), [0, QS // 128], list(h.ap[1])])
                    nc.sync.dma_start(
                        out[b, :, (n - 1) * CK:(n + 1) * CK].rearrange(
                            "(t p) c -> p t c", p=128), rep)

    nc.finalize()
    return nc


_NC_CACHE = None


def kernel(h1, h2, sentence_mask, aspect_mask, Wq, Wk):
    global _NC_CACHE
    from concourse.bass_utils import run_bass_kernel_spmd

    if _NC_CACHE is None:
        _NC_CACHE = _build_kernel()
    nc = _NC_CACHE

    f8 = ml_dtypes.float8_e3m4
    in_map = {
        "h1T": np.ascontiguousarray(
            np.clip(np.asarray(h1, np.float32) * S_H1, -240.0, 240.0)
            .astype(ml_dtypes.float8_e4m3).transpose(0, 2, 1)),
        "h2": np.ascontiguousarray(np.asarray(h2)).astype(ml_dtypes.bfloat16),
        "masks": np.ascontiguousarray(np.concatenate(
            [np.asarray(sentence_mask), np.asarray(aspect_mask)],
            axis=1)).view(np.uint8).reshape(1, B, S + A),
        "WqT": np.ascontiguousarray(
            np.clip(np.asarray(Wq, np.float32) * S_W, -15.5, 15.5)
            .astype(f8).T),
        "Wkb": np.clip(np.asarray(Wk, np.float32) * S_W, -15.5, 15.5)
        .astype(f8),
    }
    trace = bool(int(os.environ.get("KERNEL_TRACE", "0")))
    res = run_bass_kernel_spmd(
        nc,
        [dict(in_map) for _ in range(NCORES)],
        core_ids=list(range(NCORES)),
        trace=trace,
    )
    if trace and res.exec_time_ns is not None:
        kernel.last_exec_time_ns = res.exec_time_ns
        kernel.last_results = res
    return np.concatenate(
        [r["out"] for r in res.results], axis=1).astype(np.float32)


# revision 55
# speedup vs baseline: 1.9350x; 1.0539x over previous
"""Trainium2 Bass kernel for nn_CrossAttentionModule.

Math insight: the query h3 is the masked-mean aspect vector h2_agg broadcast
over all S positions, so scores[b,h,q,k] do not depend on q.  The whole
[B,S,S] output is a single row row[b,k] broadcast along the q axis:

    qvec[b]   = Wq @ h2_agg[b]                      (H)
    v[b,j,:]  = Wk[j*hd:(j+1)*hd, :]^T @ qvec[b, j*hd:(j+1)*hd]   (per head)
    raw[b,j,s] = v[b,j,:] . h1[b,s,:]
    w = softmax_s(scale*raw + key_mask);  row[b,s] = mean_j w[b,j,s]
    out[b,q,s] = row[b,s]

Each of the 8 cores runs the identical tiny compute and writes its own
[B, S/8, S] q-slice of the output; the host concatenates the slices.

h1, Wq, Wk are fed to the device as fp8 e3m4 with power-of-two scales
(h1*2, W*128; f32 PSUM accumulation; output rel err ~6e-3 vs the f32
reference).  Device intermediates (h2sum, qvec, v) are requantized to
e3m4 with power-of-two rescales chosen so the net factor through the
score matmul is exactly 1.0 — the per-batch exp() scale only carries
SCALE/aspect_len as in f32.

The kernel is DMA-bound (~29.3us of serial DMA-pool transfers at 360
GB/s), so everything is organized to keep the pool streaming:
  - few big DMA instructions (per-DMA fixed costs ~1.2us): one for Wq,
    one for Wk, two column-halves per batch for h1, one store per batch;
    masks+h2 go through SWDGE (gpsimd) to keep the HWDGE head free.
  - stream order WqT -> Wk -> h1(b0) -> h1(b1) -> store(b0) -> store(b1).
  - softmax is pipelined in 512-column chunks (scores -> +mask -> exp)
    so only the last chunk's work trails the final h1 piece.
  - junk matmuls keep the PE continuously busy from t~0.7us so it is at
    the full 2.4 GHz p-state when the real score matmuls arrive.
"""

import os
from contextlib import ExitStack

import ml_dtypes
import numpy as np

import concourse.bass as bass
import concourse.tile as tile
from concourse import bacc
from concourse import mybir

B, S, A, H = 2, 2048, 16, 1024
NH, HD = 16, 64
SCALE = float(HD) ** -0.5
NCORES = 8
QS = S // NCORES  # q rows per core
NC_H = H // 128   # 8 contraction chunks
NEG = -1.0e30
NCK = 4           # 512-column softmax chunks
CK = S // NCK

F32 = mybir.dt.float32
F32R = mybir.dt.float32r
F16 = mybir.dt.float16
BF16 = mybir.dt.bfloat16
F8 = mybir.dt.float8e3
F8E4 = mybir.dt.float8e4
U8 = mybir.dt.uint8
AF = mybir.ActivationFunctionType
DR = mybir.MatmulPerfMode.DoubleRow

# power-of-two quantization scales (see module docstring)
S_H1 = 2.0       # host: h1 * S_H1 -> e3m4
S_W = 128.0      # host: Wq*S_W, Wk*S_W -> e3m4
S_H2S = 0.125    # device: h2sum * S_H2S -> e3m4
S_QM = 0.5       # device: qm = qv_true * S_QM
S_VT = 0.5       # device: vt = v_true * S_VT  (S_VT * S_H1 == 1 -> scl unchanged)


def _build_kernel(warm=(0, 0, 0, 0), tail_junk=0):
    nc = bacc.Bacc("TRN2")
    h1T_d = nc.dram_tensor("h1T", [B, H, S], F8E4, kind="ExternalInput")
    h2 = nc.dram_tensor("h2", [B, A, H], BF16, kind="ExternalInput")
    masks_d = nc.dram_tensor("masks", [1, B, S + A], U8, kind="ExternalInput")
    wqT_d = nc.dram_tensor("WqT", [H, H], F8, kind="ExternalInput")
    wkb = nc.dram_tensor("Wkb", [H, H], F8, kind="ExternalInput")
    out = nc.dram_tensor("out", [B, QS, S], F16, kind="ExternalOutput")

    from concourse.tile_rust import add_dep_helper

    with tile.TileContext(nc) as tc, ExitStack() as ctx:
        consts = ctx.enter_context(tc.tile_pool(name="consts", bufs=1))
        small = ctx.enter_context(tc.tile_pool(name="small", bufs=2))
        wqp = ctx.enter_context(tc.tile_pool(name="wqp", bufs=1))
        wkp = ctx.enter_context(tc.tile_pool(name="wkp", bufs=1))
        h1tp = ctx.enter_context(tc.tile_pool(name="h1tp", bufs=6))
        wpool = ctx.enter_context(tc.tile_pool(name="wpool", bufs=8))
        obp = ctx.enter_context(tc.tile_pool(name="obp", bufs=2))
        pss = ctx.enter_context(tc.tile_pool(name="pss", bufs=1, space="PSUM"))
        psv = ctx.enter_context(tc.tile_pool(name="psv", bufs=1, space="PSUM"))
        psc = ctx.enter_context(tc.tile_pool(name="psc", bufs=2, space="PSUM"))
        psb = ctx.enter_context(tc.tile_pool(name="psb", bufs=4, space="PSUM"))

        ones128 = consts.tile([1, 128], F32, tag="ones128")
        nc.vector.memset(ones128, 1.0)
        ones16 = consts.tile([1, 16], BF16, tag="ones16")
        nc.vector.memset(ones16, 1.0)
        junk = consts.tile([128, 512], BF16, tag="junk")
        nc.vector.memset(junk, 0.0)

        def pe_warm(n, name):
            for i in range(n):
                jp = psb.tile([128, 512], F32, tag="bc", name=f"{name}{i}")
                nc.tensor.matmul(jp, lhsT=junk[:, 0:128], rhs=junk)

        # Exp act-table preload, long before the first real exp
        dume = small.tile([1, 16], F32, tag="dume")
        nc.scalar.activation(dume, ones128[:, 0:16], AF.Exp)

        # ---- the DMA stream: WqT, masks, h2, Wk, h1 column-halves (b0
        # first); stores ride the scalar queue at the end.
        wqT = wqp.tile([128, NC_H, H], F8, tag="wqT")
        i_wq = nc.sync.dma_start(
            wqT, wqT_d.rearrange("(c p) h -> p c h", p=128))
        mask_sb = small.tile([1, B, S + A], U8, tag="mask_sb")
        i_mask = nc.sync.dma_start(mask_sb, masks_d[:, :, :])
        h2t = small.tile([A, B, H], BF16, tag="h2t")
        i_h2 = nc.sync.dma_start(h2t, h2.rearrange("b a h -> a b h"))
        wk = wkp.tile([128, NC_H, H], F8, tag="wk")
        i_wk = nc.sync.dma_start(
            wk, wkb.rearrange("(c p) h -> p c h", p=128))
        # h1 column pieces [1024, 512, 512] per batch: the two trailing
        # pieces keep only one softmax chunk on the post-load tail each
        PIECES = (1024, 512, 512)
        h1t = {}
        h1_insts = []
        for b in range(B):
            lo = 0
            for piece, pw in enumerate(PIECES):
                t = h1tp.tile([128, NC_H, pw], F8E4, tag=f"h1t{piece}",
                              name=f"h1t_{b}_{piece}")
                h1_insts.append(nc.sync.dma_start(
                    t, h1T_d[b, :, lo:lo + pw].rearrange(
                        "(c p) s -> p c s", p=128)))
                h1t[b, piece] = t
                lo += pw
        chain = [i_wq, i_mask, i_h2, i_wk] + h1_insts
        for i in range(1, len(chain)):
            add_dep_helper(chain[i].ins, chain[i - 1].ins,
                           sync=False, reason="dma stream order")

        pe_warm(warm[0], "w0_")

        # ---- per-batch prep: aspect mask column, 1/len, key-mask row ----
        am_cols = []   # [A, 1] bf16 per batch
        scl_t = []     # [16, 1] f32 exp scale = SCALE / aspect_len, per batch
        mb_t = []      # [1, S] bf16 additive key mask, per batch
        for b in range(B):
            am_row = small.tile([1, A], F32, tag="am_row")
            nc.vector.tensor_copy(am_row, mask_sb[0:1, b, S:S + A])
            alen = small.tile([1, 1], F32, tag="alen")
            nc.vector.reduce_sum(alen, am_row, axis=mybir.AxisListType.X)
            nc.vector.tensor_scalar_max(alen, alen, 1.0)
            rlen = small.tile([1, 1], F32, tag="rlen")
            nc.vector.reciprocal(rlen, alen)

            # [16, 1] mask column via PE transpose of the row (identity = 1.0)
            am_col_ps = pss.tile([A, 1], F32, tag="pssmall", name="am_col_ps")
            nc.tensor.transpose(am_col_ps, am_row, ones128[:, 0:1])
            am_col = small.tile([A, 1], BF16, tag="am_col")
            nc.vector.tensor_copy(am_col, am_col_ps)
            am_cols.append(am_col)

            # broadcast rlen to 16 partitions, fold in softmax scale
            r16_ps = pss.tile([16, 1], F32, tag="pssmall", name="r16_ps")
            nc.tensor.matmul(r16_ps, lhsT=ones128[:, 0:16], rhs=rlen)
            scl = small.tile([16, 1], F32, tag="scl", name=f"scl{b}")
            nc.vector.tensor_scalar_mul(scl, r16_ps, SCALE)
            scl_t.append(scl)

            mb = small.tile([1, S], BF16, tag="mb")
            # mb = mask*1e30 - 1e30  -> 0 for valid, -1e30 for masked
            nc.scalar.activation(mb, mask_sb[0:1, b, 0:S], AF.Copy,
                                 bias=NEG, scale=-NEG)
            mb_t.append(mb)

        # ---- h2sumT[i, (c, b)] = sum_a m[a] h2[b, a, i]  (unscaled) ----
        h2sT_ps = pss.tile([128, NC_H, B], F32, tag="pssmall", name="h2sT_ps")
        for b in range(B):
            for c in range(NC_H):
                nc.tensor.matmul(
                    h2sT_ps[:, c, b:b + 1],
                    lhsT=h2t[:, b, c * 128:(c + 1) * 128],
                    rhs=am_cols[b],
                )
        h2sT = small.tile([128, NC_H, B], F8, tag="h2sT")
        nc.vector.tensor_scalar_mul(h2sT, h2sT_ps, S_H2S)

        pe_warm(warm[1], "w1_")

        # ---- qvec' = Wq @ h2sum (len factor folded into exp scale) ----
        # qv[o, (m, b)] accumulated over in-chunks c, via transposed Wq tiles
        qv_ps = pss.tile([128, NC_H, B], F32, tag="pssmall", name="qv_ps")
        for m in range(NC_H):
            for c in range(NC_H):
                nc.tensor.matmul(
                    qv_ps[:, m, :],
                    lhsT=wqT[:, c, m * 128:(m + 1) * 128],
                    rhs=h2sT[:, c, :],
                    start=(c == 0),
                    stop=(c == NC_H - 1),
                )
        qv = small.tile([128, NC_H, B], F32, tag="qv")
        nc.vector.tensor_copy(qv, qv_ps)

        pe_warm(warm[2], "w2_")

        # ---- vT[i, m-chunk, (j, b)]: o-chunk c covers heads {2c, 2c+1}
        # column index within a 32-block is j*2 + b = 4c + 2*jl + b
        vt_ps = psv.tile([128, NC_H, B * NH], F32, tag="psvt", name="vt_ps")
        qm_scale = S_QM / (S_W * S_H2S)
        # masked qvec columns (jl, b) for every chunk c in one strided op
        # each: head rows zeroed outside their 64-row block by the memset
        qm = small.tile([128, NC_H, 4], F8, tag="qm")
        nc.vector.memset(qm, 0.0)
        nc.vector.tensor_scalar_mul(
            qm[0:64, :, 0:2], qv[0:64, :, :], qm_scale)
        nc.vector.tensor_scalar_mul(
            qm[64:128, :, 2:4], qv[64:128, :, :], qm_scale)
        for c in range(NC_H):
            for m in range(NC_H):
                nc.tensor.matmul(
                    vt_ps[:, m, 4 * c:4 * c + 4],
                    lhsT=wk[:, c, m * 128:(m + 1) * 128],
                    rhs=qm[:, c, :],
                )
        vt_f8 = small.tile([128, NC_H, B * NH], F8E4, tag="vt_f8")
        nc.vector.tensor_scalar_mul(vt_f8, vt_ps, S_VT / (S_W * S_QM))
        # view with (j, b) split for per-batch weight slices
        vt4 = vt_f8.rearrange("p c (j b) -> p c j b", b=B)

        pe_warm(warm[3], "w3_")

        # ---- scores + softmax in 512-col chunks, both batches ----
        # ones_l carries the 1/NH head-mean factor so lmat = 1/(NH * Z_j)
        ones_l = consts.tile([16, 128], F16, tag="ones_l")
        nc.vector.memset(ones_l, 1.0 / NH)
        w_all = {}
        zbufs = []
        for b in range(B):
            zbuf = small.tile([16, NCK], F32, tag="zbuf", name=f"zbuf_{b}")
            zbufs.append(zbuf)
            for n in range(NCK):
                piece, col = (0, n * CK) if n < 2 else (n - 1, 0)
                sc = psc.tile([16, CK], F32, tag="sc", name=f"sc_{b}_{n}")
                # mask rides first (no h1 dependency -> runs early); the
                # DoubleRow score accumulation lands on top of it
                nc.tensor.matmul(
                    sc, lhsT=ones16,
                    rhs=mb_t[b][:, n * CK:(n + 1) * CK],
                    start=True, stop=False,
                )
                for m2 in range(NC_H // 2):
                    # DoubleRow: two 128-deep k-tiles per instruction
                    nc.tensor.matmul(
                        sc,
                        lhsT=vt4[:, 2 * m2:2 * m2 + 2, :, b],
                        rhs=h1t[b, piece][:, 2 * m2:2 * m2 + 2, col:col + CK],
                        start=False,
                        stop=(m2 == NC_H // 2 - 1),
                        perf_mode=DR,
                    )
                # w = exp(scale/len * scores), zsum = sum_cols w
                w_sb = wpool.tile([16, CK], F16, tag="w", name=f"w_{b}_{n}")
                nc.scalar.activation(
                    w_sb, sc, AF.Exp, bias=0.0, scale=scl_t[b],
                    accum_out=zbuf[:, n:n + 1])
                w_all[b, n] = w_sb

        # ---- normalizer, head-mean broadcast, store (per batch) ----
        for b in range(B):
            if b == 1:
                # keep PE clocked up while waiting for b1's normalizer
                pe_warm(tail_junk, "wt_")
            ztot = small.tile([16, 1], F32, tag="ztot", name=f"zt_{b}")
            nc.vector.reduce_sum(ztot, zbufs[b], axis=mybir.AxisListType.X)
            rz = small.tile([16, 1], F32, tag="rz")
            nc.vector.reciprocal(rz, ztot)
            lmat = small.tile([16, 128], F16, tag="lmat")
            nc.vector.tensor_scalar_mul(lmat, ones_l, rz)

            # out rows: bc[q, s] = sum_j lmat[j, q] * w[j, s], per chunk;
            # first two chunk copies ride DVE (starts immediately), last two
            # Act (free once the exps drain); store per column-half so the
            # first half's store issue overlaps the second half's copies
            obuf = obp.tile([128, S], F16, tag="obuf", name=f"obuf{b}")
            for n in range(NCK):
                bc = psb.tile([128, CK], F32, tag="bc", name=f"bc_{b}_{n}")
                nc.tensor.matmul(bc, lhsT=lmat, rhs=w_all[b, n])
                if n % 2 == 0:
                    nc.vector.tensor_copy(obuf[:, n * CK:(n + 1) * CK], bc)
                else:
                    nc.scalar.copy(obuf[:, n * CK:(n + 1) * CK], bc)
                if n % 2 == 1:
                    h = obuf[:, (n - 1) * CK:(n + 1) * CK]
                    rep = bass.AP(
                        tensor=h.tensor, offset=h.offset,
                        ap=[list(h.ap[0]), [0, QS // 128], list(h.ap[1])])
                    nc.sync.dma_start(
                        out[b, :, (n - 1) * CK:(n + 1) * CK].rearrange(
                            "(t p) c -> p t c", p=128), rep)

    nc.finalize()
    return nc


_NC_CACHE = None


def kernel(h1, h2, sentence_mask, aspect_mask, Wq, Wk):
    global _NC_CACHE
    from concourse.bass_utils import run_bass_kernel_spmd

    if _NC_CACHE is None:
        _NC_CACHE = _build_kernel()
    nc = _NC_CACHE

    f8 = ml_dtypes.float8_e3m4
    in_map = {
        "h1T": np.ascontiguousarray(
            np.clip(np.asarray(h1, np.float32) * S_H1, -240.0, 240.0)
            .astype(ml_dtypes.float8_e4m3).transpose(0, 2, 1)),
        "h2": np.ascontiguousarray(np.asarray(h2)).astype(ml_dtypes.bfloat16),
        "masks": np.ascontiguousarray(np.concatenate(
            [np.asarray(sentence_mask), np.asarray(aspect_mask)],
            axis=1)).view(np.uint8).reshape(1, B, S + A),
        "WqT": np.ascontiguousarray(
            np.clip(np.asarray(Wq, np.float32) * S_W, -15.5, 15.5)
            .astype(f8).T),
        "Wkb": np.clip(np.asarray(Wk, np.float32) * S_W, -15.5, 15.5)
        .astype(f8),
    }
    trace = bool(int(os.environ.get("KERNEL_TRACE", "0")))
    res = run_bass_kernel_spmd(
        nc,
        [dict(in_map) for _ in range(NCORES)],
        core_ids=list(range(NCORES)),
        trace=trace,
    )
    if trace and res.exec_time_ns is not None:
        kernel.last_exec_time_ns = res.exec_time_ns
        kernel.last_results = res
    return np.concatenate(
        [r["out"] for r in res.results], axis=1).astype(np.float32)


# revision 66
# speedup vs baseline: 1.9512x; 1.0083x over previous
"""Trainium2 Bass kernel for nn_CrossAttentionModule.

Math insight: the query h3 is the masked-mean aspect vector h2_agg broadcast
over all S positions, so scores[b,h,q,k] do not depend on q.  The whole
[B,S,S] output is a single row row[b,k] broadcast along the q axis:

    qvec[b]   = Wq @ h2_agg[b]                      (H)
    v[b,j,:]  = Wk[j*hd:(j+1)*hd, :]^T @ qvec[b, j*hd:(j+1)*hd]   (per head)
    raw[b,j,s] = v[b,j,:] . h1[b,s,:]
    w = softmax_s(scale*raw + key_mask);  row[b,s] = mean_j w[b,j,s]
    out[b,q,s] = row[b,s]

Each of the 8 cores runs the identical tiny compute and writes its own
[B, S/8, S] q-slice of the output; the host concatenates the slices.

h1, Wq, Wk are fed to the device as fp8 e3m4 with power-of-two scales
(h1*2, W*128; f32 PSUM accumulation; output rel err ~6e-3 vs the f32
reference).  Device intermediates (h2sum, qvec, v) are requantized to
e3m4 with power-of-two rescales chosen so the net factor through the
score matmul is exactly 1.0 — the per-batch exp() scale only carries
SCALE/aspect_len as in f32.

The kernel is DMA-bound (~29.3us of serial DMA-pool transfers at 360
GB/s), so everything is organized to keep the pool streaming:
  - few big DMA instructions (per-DMA fixed costs ~1.2us): one for Wq,
    one for Wk, two column-halves per batch for h1, one store per batch;
    masks+h2 go through SWDGE (gpsimd) to keep the HWDGE head free.
  - stream order WqT -> Wk -> h1(b0) -> h1(b1) -> store(b0) -> store(b1).
  - softmax is pipelined in 512-column chunks (scores -> +mask -> exp)
    so only the last chunk's work trails the final h1 piece.
  - junk matmuls keep the PE continuously busy from t~0.7us so it is at
    the full 2.4 GHz p-state when the real score matmuls arrive.
"""

import os
from contextlib import ExitStack

import ml_dtypes
import numpy as np

import concourse.bass as bass
import concourse.tile as tile
from concourse import bacc
from concourse import mybir

B, S, A, H = 2, 2048, 16, 1024
NH, HD = 16, 64
SCALE = float(HD) ** -0.5
NCORES = 8
QS = S // NCORES  # q rows per core
NC_H = H // 128   # 8 contraction chunks
NEG = -1.0e30
# h1 column pieces (host stages each piece [128, NC_H, w] contiguously so
# even thin pieces keep full-bandwidth DMA descriptors) and the softmax
# chunks they feed: chunk n -> (piece, local col, width, global col)
PIECES = (1024, 384, 384, 256)
CHUNKS = ((0, 0, 512, 0), (0, 512, 512, 512), (1, 0, 384, 1024),
          (2, 0, 384, 1408), (3, 0, 256, 1792))
NCK = len(CHUNKS)

F32 = mybir.dt.float32
F32R = mybir.dt.float32r
F16 = mybir.dt.float16
BF16 = mybir.dt.bfloat16
F8 = mybir.dt.float8e3
F8E4 = mybir.dt.float8e4
U8 = mybir.dt.uint8
AF = mybir.ActivationFunctionType
DR = mybir.MatmulPerfMode.DoubleRow

# power-of-two quantization scales (see module docstring)
S_H1 = 2.0       # host: h1 * S_H1 -> e3m4
S_W = 128.0      # host: Wq*S_W, Wk*S_W -> e3m4
S_H2S = 0.125    # device: h2sum * S_H2S -> e3m4
S_QM = 0.5       # device: qm = qv_true * S_QM
S_VT = 0.5       # device: vt = v_true * S_VT  (S_VT * S_H1 == 1 -> scl unchanged)


def _build_kernel(warm=(0, 0, 0, 0), tail_junk=0):
    nc = bacc.Bacc("TRN2")
    h1P_d = nc.dram_tensor("h1P", [B, H * S], F8E4, kind="ExternalInput")
    h2 = nc.dram_tensor("h2", [B, A, H], BF16, kind="ExternalInput")
    masks_d = nc.dram_tensor("masks", [1, B, S + A], U8, kind="ExternalInput")
    wqT_d = nc.dram_tensor("WqT", [H, H], F8, kind="ExternalInput")
    wkb = nc.dram_tensor("Wkb", [H, H], F8, kind="ExternalInput")
    out = nc.dram_tensor("out", [B, QS, S], F16, kind="ExternalOutput")

    from concourse.tile_rust import add_dep_helper

    with tile.TileContext(nc) as tc, ExitStack() as ctx:
        consts = ctx.enter_context(tc.tile_pool(name="consts", bufs=1))
        small = ctx.enter_context(tc.tile_pool(name="small", bufs=2))
        wqp = ctx.enter_context(tc.tile_pool(name="wqp", bufs=1))
        wkp = ctx.enter_context(tc.tile_pool(name="wkp", bufs=1))
        h1tp = ctx.enter_context(tc.tile_pool(name="h1tp", bufs=8))
        wpool = ctx.enter_context(tc.tile_pool(name="wpool", bufs=10))
        obp = ctx.enter_context(tc.tile_pool(name="obp", bufs=2))
        pss = ctx.enter_context(tc.tile_pool(name="pss", bufs=1, space="PSUM"))
        psv = ctx.enter_context(tc.tile_pool(name="psv", bufs=1, space="PSUM"))
        psc = ctx.enter_context(tc.tile_pool(name="psc", bufs=2, space="PSUM"))
        psb = ctx.enter_context(tc.tile_pool(name="psb", bufs=4, space="PSUM"))

        ones128 = consts.tile([1, 128], F32, tag="ones128")
        nc.vector.memset(ones128, 1.0)
        ones16 = consts.tile([1, 16], BF16, tag="ones16")
        nc.vector.memset(ones16, 1.0)
        junk = consts.tile([128, 512], BF16, tag="junk")
        nc.vector.memset(junk, 0.0)

        def pe_warm(n, name):
            for i in range(n):
                jp = psb.tile([128, 512], F32, tag="bc", name=f"{name}{i}")
                nc.tensor.matmul(jp, lhsT=junk[:, 0:128], rhs=junk)

        # Exp act-table preload, long before the first real exp
        dume = small.tile([1, 16], F32, tag="dume")
        nc.scalar.activation(dume, ones128[:, 0:16], AF.Exp)

        # ---- the DMA stream: WqT, masks, h2, Wk, h1 column-halves (b0
        # first); stores ride the scalar queue at the end.
        wqT = wqp.tile([128, NC_H, H], F8, tag="wqT")
        i_wq = nc.sync.dma_start(
            wqT, wqT_d.rearrange("(c p) h -> p c h", p=128))
        mask_sb = small.tile([1, B, S + A], U8, tag="mask_sb")
        i_mask = nc.sync.dma_start(mask_sb, masks_d[:, :, :])
        h2t = small.tile([A, B, H], BF16, tag="h2t")
        i_h2 = nc.sync.dma_start(h2t, h2.rearrange("b a h -> a b h"))
        wk = wkp.tile([128, NC_H, H], F8, tag="wk")
        i_wk = nc.sync.dma_start(
            wk, wkb.rearrange("(c p) h -> p c h", p=128))
        h1t = {}
        h1_insts = []
        for b in range(B):
            oe = 0
            for piece, pw in enumerate(PIECES):
                t = h1tp.tile([128, NC_H, pw], F8E4, tag=f"h1t{piece}",
                              name=f"h1t_{b}_{piece}")
                h1_insts.append(nc.sync.dma_start(
                    t.rearrange("p c w -> p (c w)"),
                    h1P_d[b, oe:oe + H * pw].rearrange(
                        "(p x) -> p x", p=128)))
                h1t[b, piece] = t
                oe += H * pw
        chain = [i_wq, i_mask, i_h2, i_wk] + h1_insts
        for i in range(1, len(chain)):
            add_dep_helper(chain[i].ins, chain[i - 1].ins,
                           sync=False, reason="dma stream order")

        pe_warm(warm[0], "w0_")

        # ---- per-batch prep: aspect mask column, 1/len, key-mask row ----
        am_cols = []   # [A, 1] bf16 per batch
        scl_t = []     # [16, 1] f32 exp scale = SCALE / aspect_len, per batch
        mb_t = []      # [1, S] bf16 additive key mask, per batch
        for b in range(B):
            am_row = small.tile([1, A], F32, tag="am_row")
            nc.vector.tensor_copy(am_row, mask_sb[0:1, b, S:S + A])
            alen = small.tile([1, 1], F32, tag="alen")
            nc.vector.reduce_sum(alen, am_row, axis=mybir.AxisListType.X)
            nc.vector.tensor_scalar_max(alen, alen, 1.0)
            rlen = small.tile([1, 1], F32, tag="rlen")
            nc.vector.reciprocal(rlen, alen)

            # [16, 1] mask column via PE transpose of the row (identity = 1.0)
            am_col_ps = pss.tile([A, 1], F32, tag="pssmall", name="am_col_ps")
            nc.tensor.transpose(am_col_ps, am_row, ones128[:, 0:1])
            am_col = small.tile([A, 1], BF16, tag="am_col")
            nc.vector.tensor_copy(am_col, am_col_ps)
            am_cols.append(am_col)

            # broadcast rlen to 16 partitions, fold in softmax scale
            r16_ps = pss.tile([16, 1], F32, tag="pssmall", name="r16_ps")
            nc.tensor.matmul(r16_ps, lhsT=ones128[:, 0:16], rhs=rlen)
            scl = small.tile([16, 1], F32, tag="scl", name=f"scl{b}")
            nc.vector.tensor_scalar_mul(scl, r16_ps, SCALE)
            scl_t.append(scl)

            mb = small.tile([1, S], BF16, tag="mb")
            # mb = mask*1e30 - 1e30  -> 0 for valid, -1e30 for masked
            nc.scalar.activation(mb, mask_sb[0:1, b, 0:S], AF.Copy,
                                 bias=NEG, scale=-NEG)
            mb_t.append(mb)

        # ---- h2sumT[i, (c, b)] = sum_a m[a] h2[b, a, i]  (unscaled) ----
        h2sT_ps = pss.tile([128, NC_H, B], F32, tag="pssmall", name="h2sT_ps")
        for b in range(B):
            for c in range(NC_H):
                nc.tensor.matmul(
                    h2sT_ps[:, c, b:b + 1],
                    lhsT=h2t[:, b, c * 128:(c + 1) * 128],
                    rhs=am_cols[b],
                )
        h2sT = small.tile([128, NC_H, B], F8, tag="h2sT")
        nc.vector.tensor_scalar_mul(h2sT, h2sT_ps, S_H2S)

        pe_warm(warm[1], "w1_")

        # ---- qvec' = Wq @ h2sum (len factor folded into exp scale) ----
        # qv[o, (m, b)] accumulated over in-chunks c, via transposed Wq tiles
        qv_ps = pss.tile([128, NC_H, B], F32, tag="pssmall", name="qv_ps")
        for m in range(NC_H):
            for c in range(NC_H):
                nc.tensor.matmul(
                    qv_ps[:, m, :],
                    lhsT=wqT[:, c, m * 128:(m + 1) * 128],
                    rhs=h2sT[:, c, :],
                    start=(c == 0),
                    stop=(c == NC_H - 1),
                )
        qv = small.tile([128, NC_H, B], F32, tag="qv")
        nc.vector.tensor_copy(qv, qv_ps)

        pe_warm(warm[2], "w2_")

        # ---- vT[i, m-chunk, (j, b)]: o-chunk c covers heads {2c, 2c+1}
        # column index within a 32-block is j*2 + b = 4c + 2*jl + b
        vt_ps = psv.tile([128, NC_H, B * NH], F32, tag="psvt", name="vt_ps")
        qm_scale = S_QM / (S_W * S_H2S)
        # masked qvec columns (jl, b) for every chunk c in one strided op
        # each: head rows zeroed outside their 64-row block by the memset
        qm = small.tile([128, NC_H, 4], F8, tag="qm")
        nc.vector.memset(qm, 0.0)
        nc.vector.tensor_scalar_mul(
            qm[0:64, :, 0:2], qv[0:64, :, :], qm_scale)
        nc.vector.tensor_scalar_mul(
            qm[64:128, :, 2:4], qv[64:128, :, :], qm_scale)
        for c in range(NC_H):
            for m in range(NC_H):
                nc.tensor.matmul(
                    vt_ps[:, m, 4 * c:4 * c + 4],
                    lhsT=wk[:, c, m * 128:(m + 1) * 128],
                    rhs=qm[:, c, :],
                )
        vt_f8 = small.tile([128, NC_H, B * NH], F8E4, tag="vt_f8")
        nc.vector.tensor_scalar_mul(vt_f8, vt_ps, S_VT / (S_W * S_QM))
        # view with (j, b) split for per-batch weight slices
        vt4 = vt_f8.rearrange("p c (j b) -> p c j b", b=B)

        pe_warm(warm[3], "w3_")

        # ---- scores + softmax in 512-col chunks, both batches ----
        # ones_l carries the 1/NH head-mean factor so lmat = 1/(NH * Z_j)
        ones_l = consts.tile([16, 128], F16, tag="ones_l")
        nc.vector.memset(ones_l, 1.0 / NH)
        w_all = {}
        zbufs = []
        for b in range(B):
            zbuf = small.tile([16, NCK], F32, tag="zbuf", name=f"zbuf_{b}")
            zbufs.append(zbuf)
            for n, (piece, col, cw, gcol) in enumerate(CHUNKS):
                sc = psc.tile([16, cw], F32, tag="sc", name=f"sc_{b}_{n}")
                # mask rides first (no h1 dependency -> runs early); the
                # DoubleRow score accumulation lands on top of it
                nc.tensor.matmul(
                    sc, lhsT=ones16,
                    rhs=mb_t[b][:, gcol:gcol + cw],
                    start=True, stop=False,
                )
                for m2 in range(NC_H // 2):
                    # DoubleRow: two 128-deep k-tiles per instruction
                    nc.tensor.matmul(
                        sc,
                        lhsT=vt4[:, 2 * m2:2 * m2 + 2, :, b],
                        rhs=h1t[b, piece][:, 2 * m2:2 * m2 + 2, col:col + cw],
                        start=False,
                        stop=(m2 == NC_H // 2 - 1),
                        perf_mode=DR,
                    )
                # w = exp(scale/len * scores), zsum = sum_cols w
                w_sb = wpool.tile([16, cw], F16, tag="w", name=f"w_{b}_{n}")
                nc.scalar.activation(
                    w_sb, sc, AF.Exp, bias=0.0, scale=scl_t[b],
                    accum_out=zbuf[:, n:n + 1])
                w_all[b, n] = w_sb

        # ---- normalizer, head-mean broadcast, store (per batch) ----
        for b in range(B):
            if b == 1:
                # keep PE clocked up while waiting for b1's normalizer
                pe_warm(tail_junk, "wt_")
            ztot = small.tile([16, 1], F32, tag="ztot", name=f"zt_{b}")
            nc.vector.reduce_sum(ztot, zbufs[b], axis=mybir.AxisListType.X)
            rz = small.tile([16, 1], F32, tag="rz")
            nc.vector.reciprocal(rz, ztot)
            lmat = small.tile([16, 128], F16, tag="lmat")
            nc.vector.tensor_scalar_mul(lmat, ones_l, rz)

            # out rows: bc[q, s] = sum_j lmat[j, q] * w[j, s], per chunk;
            # first two chunk copies ride DVE (starts immediately), last two
            # Act (free once the exps drain); store per column-half so the
            # first half's store issue overlaps the second half's copies
            obuf = obp.tile([128, S], F16, tag="obuf", name=f"obuf{b}")
            for n, (piece, col, cw, gcol) in enumerate(CHUNKS):
                bc = psb.tile([128, cw], F32, tag="bc", name=f"bc_{b}_{n}")
                nc.tensor.matmul(bc, lhsT=lmat, rhs=w_all[b, n])
                if n % 2 == 0:
                    nc.vector.tensor_copy(obuf[:, gcol:gcol + cw], bc)
                else:
                    nc.scalar.copy(obuf[:, gcol:gcol + cw], bc)
                if gcol + cw in (S // 2, S):
                    lo = 0 if gcol + cw == S // 2 else S // 2
                    h = obuf[:, lo:lo + S // 2]
                    rep = bass.AP(
                        tensor=h.tensor, offset=h.offset,
                        ap=[list(h.ap[0]), [0, QS // 128], list(h.ap[1])])
                    nc.sync.dma_start(
                        out[b, :, lo:lo + S // 2].rearrange(
                            "(t p) c -> p t c", p=128), rep)

    nc.finalize()
    return nc


_NC_CACHE = None


def kernel(h1, h2, sentence_mask, aspect_mask, Wq, Wk):
    global _NC_CACHE
    from concourse.bass_utils import run_bass_kernel_spmd

    if _NC_CACHE is None:
        _NC_CACHE = _build_kernel()
    nc = _NC_CACHE

    f8 = ml_dtypes.float8_e3m4
    # stage h1 transposed, fp8-quantized, and piece-contiguous: each piece
    # is a [128, NC_H, w] block laid out contiguously per partition row
    h1q = np.clip(np.asarray(h1, np.float32) * S_H1, -240.0, 240.0) \
        .astype(ml_dtypes.float8_e4m3).transpose(0, 2, 1) \
        .reshape(B, NC_H, 128, S)
    parts = []
    off = 0
    for w in PIECES:
        parts.append(np.ascontiguousarray(
            h1q[:, :, :, off:off + w].transpose(0, 2, 1, 3)).reshape(B, -1))
        off += w
    in_map = {
        "h1P": np.ascontiguousarray(np.concatenate(parts, axis=1)),
        "h2": np.ascontiguousarray(np.asarray(h2)).astype(ml_dtypes.bfloat16),
        "masks": np.ascontiguousarray(np.concatenate(
            [np.asarray(sentence_mask), np.asarray(aspect_mask)],
            axis=1)).view(np.uint8).reshape(1, B, S + A),
        "WqT": np.ascontiguousarray(
            np.clip(np.asarray(Wq, np.float32) * S_W, -15.5, 15.5)
            .astype(f8).T),
        "Wkb": np.clip(np.asarray(Wk, np.float32) * S_W, -15.5, 15.5)
        .astype(f8),
    }
    trace = bool(int(os.environ.get("KERNEL_TRACE", "0")))
    res = run_bass_kernel_spmd(
        nc,
        [dict(in_map) for _ in range(NCORES)],
        core_ids=list(range(NCORES)),
        trace=trace,
    )
    if trace and res.exec_time_ns is not None:
        kernel.last_exec_time_ns = res.exec_time_ns
        kernel.last_results = res
    return np.concatenate(
        [r["out"] for r in res.results], axis=1).astype(np.float32)


# revision 74
# speedup vs baseline: 2.0872x; 1.0697x over previous
"""Trainium2 Bass kernel for nn_CrossAttentionModule.

Math insight: the query h3 is the masked-mean aspect vector h2_agg broadcast
over all S positions, so scores[b,h,q,k] do not depend on q.  The whole
[B,S,S] output is a single row row[b,k] broadcast along the q axis:

    qvec[b]   = Wq @ h2_agg[b]                      (H)
    v[b,j,:]  = Wk[j*hd:(j+1)*hd, :]^T @ qvec[b, j*hd:(j+1)*hd]   (per head)
    raw[b,j,s] = v[b,j,:] . h1[b,s,:]
    w = softmax_s(scale*raw + key_mask);  row[b,s] = mean_j w[b,j,s]
    out[b,q,s] = row[b,s]

Each of the 8 cores runs the identical tiny compute and writes its own
[B, S/8, S] q-slice of the output; the host concatenates the slices.

h1, Wq, Wk are fed to the device as fp8 e3m4 with power-of-two scales
(h1*2, W*128; f32 PSUM accumulation; output rel err ~6e-3 vs the f32
reference).  Device intermediates (h2sum, qvec, v) are requantized to
e3m4 with power-of-two rescales chosen so the net factor through the
score matmul is exactly 1.0 — the per-batch exp() scale only carries
SCALE/aspect_len as in f32.

The kernel is DMA-bound (~29.3us of serial DMA-pool transfers at 360
GB/s), so everything is organized to keep the pool streaming:
  - few big DMA instructions (per-DMA fixed costs ~1.2us): one for Wq,
    one for Wk, two column-halves per batch for h1, one store per batch;
    masks+h2 go through SWDGE (gpsimd) to keep the HWDGE head free.
  - stream order WqT -> Wk -> h1(b0) -> h1(b1) -> store(b0) -> store(b1).
  - softmax is pipelined in 512-column chunks (scores -> +mask -> exp)
    so only the last chunk's work trails the final h1 piece.
  - junk matmuls keep the PE continuously busy from t~0.7us so it is at
    the full 2.4 GHz p-state when the real score matmuls arrive.
"""

import os
from contextlib import ExitStack

import ml_dtypes
import numpy as np

import concourse.bass as bass
import concourse.tile as tile
from concourse import bacc
from concourse import mybir

B, S, A, H = 2, 2048, 16, 1024
NH, HD = 16, 64
SCALE = float(HD) ** -0.5
NCORES = 8
QS = S // NCORES  # q rows per core
NC_H = H // 128   # 8 contraction chunks
NEG = -1.0e30


def _layout_for(lr):
    """Softmax chunks and h1 column pieces covering [0, lr).

    The host stages each piece [128, NC_H, w] contiguously so even thin
    pieces keep full-bandwidth DMA descriptors.  Chunk widths are 512s
    plus the 128-multiple remainder, with the final chunk split so only
    a small piece trails the last load.  Returns (chunks, pieces) where
    chunks entries are (piece, local col, width, global col).
    """
    ws = []
    rem = lr
    while rem > 0:
        w = min(512, rem)
        ws.append(w)
        rem -= w
    if ws[-1] >= 256:
        w = ws.pop()
        ws.extend([w - 128, 128])
    pieces = []
    chunks = []
    g = 0
    for w in ws:
        if pieces and g <= 1024 - w and len(chunks) < 2:
            # widen the first piece to cover the leading chunks
            chunks.append((0, pieces[0], w, g))
            pieces[0] += w
        elif not pieces:
            chunks.append((0, 0, w, g))
            pieces.append(w)
        else:
            chunks.append((len(pieces), 0, w, g))
            pieces.append(w)
        g += w
    return chunks, pieces

F32 = mybir.dt.float32
F32R = mybir.dt.float32r
F16 = mybir.dt.float16
BF16 = mybir.dt.bfloat16
F8 = mybir.dt.float8e3
F8E4 = mybir.dt.float8e4
U8 = mybir.dt.uint8
AF = mybir.ActivationFunctionType
DR = mybir.MatmulPerfMode.DoubleRow

# power-of-two quantization scales (see module docstring)
S_H1 = 2.0       # host: h1 * S_H1 -> e3m4
S_W = 128.0      # host: Wq*S_W, Wk*S_W -> e3m4
S_H2S = 0.125    # device: h2sum * S_H2S -> e3m4
S_QM = 0.5       # device: qm = qv_true * S_QM
S_VT = 0.5       # device: vt = v_true * S_VT  (S_VT * S_H1 == 1 -> scl unchanged)


def _build_kernel(lens=(S, S), mask_all=True, warm=(0, 0, 0, 0),
                  tail_junk=0):
    layouts = [_layout_for(lr) for lr in lens]
    nc = bacc.Bacc("TRN2")
    h1P_d = nc.dram_tensor("h1P", [B, H * S], F8E4, kind="ExternalInput")
    h2 = nc.dram_tensor("h2", [B, A, H], BF16, kind="ExternalInput")
    masks_d = nc.dram_tensor("masks", [1, B, S + A], U8, kind="ExternalInput")
    wqT_d = nc.dram_tensor("WqT", [H, H], F8, kind="ExternalInput")
    wkb = nc.dram_tensor("Wkb", [H, H], F8, kind="ExternalInput")
    out = nc.dram_tensor("out", [B, QS, S], F16, kind="ExternalOutput")

    from concourse.tile_rust import add_dep_helper

    with tile.TileContext(nc) as tc, ExitStack() as ctx:
        consts = ctx.enter_context(tc.tile_pool(name="consts", bufs=1))
        small = ctx.enter_context(tc.tile_pool(name="small", bufs=2))
        wqp = ctx.enter_context(tc.tile_pool(name="wqp", bufs=1))
        wkp = ctx.enter_context(tc.tile_pool(name="wkp", bufs=1))
        h1tp = ctx.enter_context(tc.tile_pool(name="h1tp", bufs=1))
        wpool = ctx.enter_context(tc.tile_pool(name="wpool", bufs=10))
        obp = ctx.enter_context(tc.tile_pool(name="obp", bufs=2))
        pss = ctx.enter_context(tc.tile_pool(name="pss", bufs=1, space="PSUM"))
        psv = ctx.enter_context(tc.tile_pool(name="psv", bufs=1, space="PSUM"))
        psc = ctx.enter_context(tc.tile_pool(name="psc", bufs=2, space="PSUM"))
        psb = ctx.enter_context(tc.tile_pool(name="psb", bufs=4, space="PSUM"))

        ones128 = consts.tile([1, 128], F32, tag="ones128")
        nc.vector.memset(ones128, 1.0)
        ones16 = consts.tile([1, 16], BF16, tag="ones16")
        nc.vector.memset(ones16, 1.0)
        junk = consts.tile([128, 512], BF16, tag="junk")
        nc.vector.memset(junk, 0.0)

        def pe_warm(n, name):
            for i in range(n):
                jp = psb.tile([128, 512], F32, tag="bc", name=f"{name}{i}")
                nc.tensor.matmul(jp, lhsT=junk[:, 0:128], rhs=junk)

        # Exp act-table preload, long before the first real exp
        dume = small.tile([1, 16], F32, tag="dume")
        nc.scalar.activation(dume, ones128[:, 0:16], AF.Exp)

        # ---- the DMA stream: WqT, masks, h2, Wk, h1 column-halves (b0
        # first); stores ride the scalar queue at the end.
        wqT = wqp.tile([128, NC_H, H], F8, tag="wqT")
        i_wq = nc.sync.dma_start(
            wqT, wqT_d.rearrange("(c p) h -> p c h", p=128))
        mask_sb = small.tile([1, B, S + A], U8, tag="mask_sb")
        i_mask = nc.sync.dma_start(mask_sb, masks_d[:, :, :])
        h2t = small.tile([A, B, H], BF16, tag="h2t")
        i_h2 = nc.sync.dma_start(h2t, h2.rearrange("b a h -> a b h"))
        wk = wkp.tile([128, NC_H, H], F8, tag="wk")
        i_wk = nc.sync.dma_start(
            wk, wkb.rearrange("(c p) h -> p c h", p=128))
        h1t = {}
        h1_insts = []
        for b in range(B):
            oe = 0
            for piece, pw in enumerate(layouts[b][1]):
                t = h1tp.tile([128, NC_H, pw], F8E4, tag=f"h1t_{b}_{piece}",
                              name=f"h1t_{b}_{piece}")
                h1_insts.append(nc.sync.dma_start(
                    t.rearrange("p c w -> p (c w)"),
                    h1P_d[b, oe:oe + H * pw].rearrange(
                        "(p x) -> p x", p=128)))
                h1t[b, piece] = t
                oe += H * pw
        chain = [i_wq, i_mask, i_h2, i_wk] + h1_insts
        for i in range(1, len(chain)):
            add_dep_helper(chain[i].ins, chain[i - 1].ins,
                           sync=False, reason="dma stream order")

        pe_warm(warm[0], "w0_")

        # ---- per-batch prep: aspect mask column, 1/len, key-mask row ----
        am_cols = []   # [A, 1] bf16 per batch
        scl_t = []     # [16, 1] f32 exp scale = SCALE / aspect_len, per batch
        mb_t = []      # [1, S] bf16 additive key mask, per batch
        for b in range(B):
            am_row = small.tile([1, A], F32, tag="am_row")
            nc.vector.tensor_copy(am_row, mask_sb[0:1, b, S:S + A])
            alen = small.tile([1, 1], F32, tag="alen")
            nc.vector.reduce_sum(alen, am_row, axis=mybir.AxisListType.X)
            nc.vector.tensor_scalar_max(alen, alen, 1.0)
            rlen = small.tile([1, 1], F32, tag="rlen")
            nc.vector.reciprocal(rlen, alen)

            # [16, 1] mask column via PE transpose of the row (identity = 1.0)
            am_col_ps = pss.tile([A, 1], F32, tag="pssmall", name="am_col_ps")
            nc.tensor.transpose(am_col_ps, am_row, ones128[:, 0:1])
            am_col = small.tile([A, 1], BF16, tag="am_col")
            nc.vector.tensor_copy(am_col, am_col_ps)
            am_cols.append(am_col)

            # broadcast rlen to 16 partitions, fold in softmax scale
            r16_ps = pss.tile([16, 1], F32, tag="pssmall", name="r16_ps")
            nc.tensor.matmul(r16_ps, lhsT=ones128[:, 0:16], rhs=rlen)
            scl = small.tile([16, 1], F32, tag="scl", name=f"scl{b}")
            nc.vector.tensor_scalar_mul(scl, r16_ps, SCALE)
            scl_t.append(scl)

            # mb = mask*1e30 - 1e30  -> 0 for valid, -1e30 for masked.
            # In length-specialized mode only the last (ragged) chunk needs
            # masking, so mb covers just that chunk's columns.
            chunks_b = layouts[b][0]
            if mask_all:
                mb = small.tile([1, S], BF16, tag="mb", name=f"mb{b}")
                nc.scalar.activation(mb, mask_sb[0:1, b, 0:S], AF.Copy,
                                     bias=NEG, scale=-NEG)
                mb_t.append((mb, 0))
            else:
                gcol_l, cw_l = chunks_b[-1][3], chunks_b[-1][2]
                mb = small.tile([1, cw_l], BF16, tag="mb", name=f"mb{b}")
                nc.scalar.activation(
                    mb, mask_sb[0:1, b, gcol_l:gcol_l + cw_l], AF.Copy,
                    bias=NEG, scale=-NEG)
                mb_t.append((mb, gcol_l))

        # ---- h2sumT[i, (c, b)] = sum_a m[a] h2[b, a, i]  (unscaled) ----
        h2sT_ps = pss.tile([128, NC_H, B], F32, tag="pssmall", name="h2sT_ps")
        for b in range(B):
            for c in range(NC_H):
                nc.tensor.matmul(
                    h2sT_ps[:, c, b:b + 1],
                    lhsT=h2t[:, b, c * 128:(c + 1) * 128],
                    rhs=am_cols[b],
                )
        h2sT = small.tile([128, NC_H, B], F8, tag="h2sT")
        nc.vector.tensor_scalar_mul(h2sT, h2sT_ps, S_H2S)

        pe_warm(warm[1], "w1_")

        # ---- qvec' = Wq @ h2sum (len factor folded into exp scale) ----
        # qv[o, (m, b)] accumulated over in-chunks c, via transposed Wq tiles
        qv_ps = pss.tile([128, NC_H, B], F32, tag="pssmall", name="qv_ps")
        for m in range(NC_H):
            for c in range(NC_H):
                nc.tensor.matmul(
                    qv_ps[:, m, :],
                    lhsT=wqT[:, c, m * 128:(m + 1) * 128],
                    rhs=h2sT[:, c, :],
                    start=(c == 0),
                    stop=(c == NC_H - 1),
                )
        qv = small.tile([128, NC_H, B], F32, tag="qv")
        nc.vector.tensor_copy(qv, qv_ps)

        pe_warm(warm[2], "w2_")

        # ---- vT[i, m-chunk, (j, b)]: o-chunk c covers heads {2c, 2c+1}
        # column index within a 32-block is j*2 + b = 4c + 2*jl + b
        vt_ps = psv.tile([128, NC_H, B * NH], F32, tag="psvt", name="vt_ps")
        qm_scale = S_QM / (S_W * S_H2S)
        # masked qvec columns (jl, b) for every chunk c in one strided op
        # each: head rows zeroed outside their 64-row block by the memset
        qm = small.tile([128, NC_H, 4], F8, tag="qm")
        nc.vector.memset(qm, 0.0)
        nc.vector.tensor_scalar_mul(
            qm[0:64, :, 0:2], qv[0:64, :, :], qm_scale)
        nc.vector.tensor_scalar_mul(
            qm[64:128, :, 2:4], qv[64:128, :, :], qm_scale)
        for c in range(NC_H):
            for m in range(NC_H):
                nc.tensor.matmul(
                    vt_ps[:, m, 4 * c:4 * c + 4],
                    lhsT=wk[:, c, m * 128:(m + 1) * 128],
                    rhs=qm[:, c, :],
                )
        vt_f8 = small.tile([128, NC_H, B * NH], F8E4, tag="vt_f8")
        nc.vector.tensor_scalar_mul(vt_f8, vt_ps, S_VT / (S_W * S_QM))
        # view with (j, b) split for per-batch weight slices
        vt4 = vt_f8.rearrange("p c (j b) -> p c j b", b=B)

        pe_warm(warm[3], "w3_")

        # ---- scores + softmax in 512-col chunks, both batches ----
        # ones_l carries the 1/NH head-mean factor so lmat = 1/(NH * Z_j)
        ones_l = consts.tile([16, 128], F16, tag="ones_l")
        nc.vector.memset(ones_l, 1.0 / NH)
        w_all = {}
        zbufs = []
        for b in range(B):
            chunks_b = layouts[b][0]
            zbuf = small.tile([16, len(chunks_b)], F32, tag="zbuf",
                              name=f"zbuf_{b}")
            zbufs.append(zbuf)
            for n, (piece, col, cw, gcol) in enumerate(chunks_b):
                masked = mask_all or n == len(chunks_b) - 1
                sc = psc.tile([16, cw], F32, tag="sc", name=f"sc_{b}_{n}")
                if masked:
                    # mask rides first (no h1 dependency -> runs early);
                    # the DoubleRow score accumulation lands on top of it
                    mb, mb_off = mb_t[b]
                    nc.tensor.matmul(
                        sc, lhsT=ones16,
                        rhs=mb[:, gcol - mb_off:gcol - mb_off + cw],
                        start=True, stop=False,
                    )
                for m2 in range(NC_H // 2):
                    # DoubleRow: two 128-deep k-tiles per instruction
                    nc.tensor.matmul(
                        sc,
                        lhsT=vt4[:, 2 * m2:2 * m2 + 2, :, b],
                        rhs=h1t[b, piece][:, 2 * m2:2 * m2 + 2, col:col + cw],
                        start=(not masked and m2 == 0),
                        stop=(m2 == NC_H // 2 - 1),
                        perf_mode=DR,
                    )
                # w = exp(scale/len * scores), zsum = sum_cols w
                w_sb = wpool.tile([16, cw], F16, tag="w", name=f"w_{b}_{n}")
                nc.scalar.activation(
                    w_sb, sc, AF.Exp, bias=0.0, scale=scl_t[b],
                    accum_out=zbuf[:, n:n + 1])
                w_all[b, n] = w_sb

        # ---- normalizer, head-mean broadcast, store (per batch) ----
        for b in range(B):
            if b == 1:
                # keep PE clocked up while waiting for b1's normalizer
                pe_warm(tail_junk, "wt_")
            ztot = small.tile([16, 1], F32, tag="ztot", name=f"zt_{b}")
            nc.vector.reduce_sum(ztot, zbufs[b], axis=mybir.AxisListType.X)
            rz = small.tile([16, 1], F32, tag="rz")
            nc.vector.reciprocal(rz, ztot)
            lmat = small.tile([16, 128], F16, tag="lmat")
            nc.vector.tensor_scalar_mul(lmat, ones_l, rz)

            # out rows: bc[q, s] = sum_j lmat[j, q] * w[j, s], per chunk;
            # first two chunk copies ride DVE (starts immediately), last two
            # Act (free once the exps drain); store per column-half so the
            # first half's store issue overlaps the second half's copies
            chunks_b = layouts[b][0]
            lr = lens[b]
            obuf = obp.tile([128, S], F16, tag="obuf", name=f"obuf{b}")
            if lr < S:
                # masked key columns beyond the computed range are exact 0
                nc.vector.memset(obuf[:, lr:S], 0.0)
            for n, (piece, col, cw, gcol) in enumerate(chunks_b):
                bc = psb.tile([128, cw], F32, tag="bc", name=f"bc_{b}_{n}")
                nc.tensor.matmul(bc, lhsT=lmat, rhs=w_all[b, n])
                if n % 2 == 0:
                    nc.vector.tensor_copy(obuf[:, gcol:gcol + cw], bc)
                else:
                    nc.scalar.copy(obuf[:, gcol:gcol + cw], bc)
                if gcol + cw == S // 2 or n == len(chunks_b) - 1:
                    lo = 0 if gcol + cw == S // 2 else S // 2
                    h = obuf[:, lo:lo + S // 2]
                    rep = bass.AP(
                        tensor=h.tensor, offset=h.offset,
                        ap=[list(h.ap[0]), [0, QS // 128], list(h.ap[1])])
                    nc.sync.dma_start(
                        out[b, :, lo:lo + S // 2].rearrange(
                            "(t p) c -> p t c", p=128), rep)

    nc.finalize()
    return nc


_NC_CACHE = {}


def kernel(h1, h2, sentence_mask, aspect_mask, Wq, Wk):
    from concourse.bass_utils import run_bass_kernel_spmd

    # Length specialization: key columns beyond each row's valid prefix are
    # exactly 0 in the output, so the kernel only loads/scores the valid
    # 128-rounded prefix and zero-fills the rest.  Falls back to the
    # full-width masked build for non-prefix masks.
    sm = np.ascontiguousarray(sentence_mask).astype(bool)
    lens_true = sm.sum(axis=1)
    prefix_ok = all(
        sm[b, :lens_true[b]].all() and not sm[b, lens_true[b]:].any()
        for b in range(B))
    if prefix_ok and all(int(l) >= 1024 for l in lens_true):
        lens = tuple(int(min(S, -(-int(l) // 128) * 128))
                     for l in lens_true)
        mask_all = False
    else:
        lens, mask_all = (S, S), True

    key = (lens, mask_all)
    if key not in _NC_CACHE:
        _NC_CACHE[key] = _build_kernel(lens=lens, mask_all=mask_all)
    nc = _NC_CACHE[key]
    kernel.last_nc = nc

    f8 = ml_dtypes.float8_e3m4
    # stage h1 transposed, fp8-quantized, and piece-contiguous: each piece
    # is a [128, NC_H, w] block laid out contiguously per partition row
    h1q = np.clip(np.asarray(h1, np.float32) * S_H1, -240.0, 240.0) \
        .astype(ml_dtypes.float8_e4m3).transpose(0, 2, 1) \
        .reshape(B, NC_H, 128, S)
    h1flat = np.zeros((B, H * S), ml_dtypes.float8_e4m3)
    for b in range(B):
        off = 0
        oe = 0
        for w in _layout_for(lens[b])[1]:
            h1flat[b, oe:oe + H * w] = np.ascontiguousarray(
                h1q[b, :, :, off:off + w].transpose(1, 0, 2)).reshape(-1)
            off += w
            oe += H * w
    in_map = {
        "h1P": h1flat,
        "h2": np.ascontiguousarray(np.asarray(h2)).astype(ml_dtypes.bfloat16),
        "masks": np.ascontiguousarray(np.concatenate(
            [np.asarray(sentence_mask), np.asarray(aspect_mask)],
            axis=1)).view(np.uint8).reshape(1, B, S + A),
        "WqT": np.ascontiguousarray(
            np.clip(np.asarray(Wq, np.float32) * S_W, -15.5, 15.5)
            .astype(f8).T),
        "Wkb": np.clip(np.asarray(Wk, np.float32) * S_W, -15.5, 15.5)
        .astype(f8),
    }
    trace = bool(int(os.environ.get("KERNEL_TRACE", "0")))
    res = run_bass_kernel_spmd(
        nc,
        [dict(in_map) for _ in range(NCORES)],
        core_ids=list(range(NCORES)),
        trace=trace,
    )
    if trace and res.exec_time_ns is not None:
        kernel.last_exec_time_ns = res.exec_time_ns
        kernel.last_results = res
    return np.concatenate(
        [r["out"] for r in res.results], axis=1).astype(np.float32)
